# revision 29
# baseline (speedup 1.0000x reference)
"""CrossMessageTokenAttention Trainium2 kernel (8 NeuronCores, SPMD).

Contract: kernel(**inputs) takes the FULL inputs of reference.setup_inputs()
and returns the FULL [32768, 1024] float32 output.

Strategy (hardcoded for T=32768, H=1024, M=64 messages, L=512, k_imp=51,
k_conn=5, 8 cores):
  - Queries (64 msgs x 51 selected tokens) are sharded by message: core c
    owns messages [8c, 8c+8) = 408 queries (padded to 512). No collectives.
  - Algebra: scores = ((Xq@Wq+bq)/32) @ Wk^T @ X^T  (bk dropped: a per-row
    shift, invariant for top-k and softmax). Values never materialized:
    attended = (sum_k attn_k X[idx_k]) @ Wv + bv, then @ Wo + bo.
  - Importance screen runs in bf16 (error ~1e-2 sigma vs a ~1.5e-1 sigma
    top-64 margin); the exact top-51 threshold is rescued in true fp32 on
    the 512 gathered candidate rows (stage 2b), emitted AFTER phase B so
    its PE work hides under vector-bound phases.
  - Q/B projections run as f32r (fast fp32 path); the dominant
    [512,1024]x[1024,32768] score matmul runs in fp8e4 DoubleRow (2 rows
    of contraction per cycle) as a SCREEN: per 512-token block, HW top-8
    (vector.max/max_index) read from PSUM. True top-5 is inside the merged
    per-row top-8 candidates (margins are ~sigma, fp8 noise ~0.04 sigma).
  - Same-message masking on candidates by global-index range compare (each
    message is one aligned 512-token block).
  - Merge of the 512 candidates: quantized-score composite key
    q*32768+gidx (exact in f32 ints <= 2^24), single max8 pass; index
    recovered by residue (robust to round-vs-trunc casts).
  - Exact rescue: gather the 8 candidate rows in f32 (bf16 rows would add
    ~0.1 abs score noise, comparable to top-5 gaps), recompute their
    scores in fp32 on the vector engine against B, rank + softmax on the
    rescored values, weighted-sum the gathered rows.
  - Device outputs per core: updates^T [1024, 512] and query indices [512]
    (408 valid). The host scatters updates into a copy of token_features.
"""
import os

os.environ.setdefault("MYCRO_LOCAL_CACHE", "1")

import numpy as np

T, H = 32768, 1024
M, L = 64, 512
K_IMP = 51
NCORES = 8
MSG_PER_CORE = M // NCORES          # 8
QC = MSG_PER_CORE * K_IMP           # 408 queries per core
QPAD = 512
NQCH = QPAD // 128                  # 4
KCH = H // 128                      # 8
NTB = T // 512                      # 64 score blocks
TC = T // NCORES                    # 4096 tokens per core block
NEG = -1.0e30

_cache = {}


def _build_program(stages=99, sub=9, reps=1):
    import concourse.bacc as bacc
    import concourse.bass as bass
    import concourse.mybir as mybir
    import concourse.tile as tile
    from concourse.masks import make_identity

    F32 = mybir.dt.float32
    F32R = mybir.dt.float32r
    BF16 = mybir.dt.bfloat16
    FP8 = mybir.dt.float8e4
    U32 = mybir.dt.uint32
    AOP = mybir.AluOpType
    ACT = mybir.ActivationFunctionType
    AXX = mybir.AxisListType.X
    DR = mybir.MatmulPerfMode.DoubleRow

    nc = bacc.Bacc(None, target_bir_lowering=False, debug=False,
                   num_devices=NCORES)

    # ---------------- DRAM I/O ----------------
    x_d = nc.declare_dram_parameter("x", [T, H], F32, isOutput=False)
    xt8_d = nc.declare_dram_parameter("xt8", [H, T], FP8, isOutput=False)
    xtcb_d = nc.declare_dram_parameter("xtcb", [H, TC], BF16, isOutput=False)
    wq_d = nc.declare_dram_parameter("wq", [H, H], F32, isOutput=False)
    wkT_d = nc.declare_dram_parameter("wkT", [H, H], F32, isOutput=False)
    wv_d = nc.declare_dram_parameter("wv", [H, H], F32R, isOutput=False)
    wo_d = nc.declare_dram_parameter("wo", [H, H], F32R, isOutput=False)
    wi1b_d = nc.declare_dram_parameter("wi1b", [H, H // 2], BF16, isOutput=False)
    wi1x_d = nc.declare_dram_parameter("wi1x", [H, H // 2], F32, isOutput=False)
    wi2r_d = nc.declare_dram_parameter("wi2r", [128, 4], F32R, isOutput=False)
    wi2s_d = nc.declare_dram_parameter("wi2s", [128, 4], F32, isOutput=False)
    bi1s_d = nc.declare_dram_parameter("bi1s", [128, 4], F32, isOutput=False)
    bq32s_d = nc.declare_dram_parameter("bq32s", [128, 8], F32, isOutput=False)
    bvs_d = nc.declare_dram_parameter("bvs", [128, 8], F32, isOutput=False)
    bos_d = nc.declare_dram_parameter("bos", [128, 8], F32, isOutput=False)
    bstg_d = nc.declare_dram_parameter("bstg", [8, 1], F32, isOutput=False)
    qlo_d = nc.declare_dram_parameter("qlo", [128, 4], F32, isOutput=False)
    qhi_d = nc.declare_dram_parameter("qhi", [128, 4], F32, isOutput=False)
    cbase_d = nc.declare_dram_parameter("cbase", [128, 512], F32, isOutput=False)

    updT_o = nc.declare_dram_parameter("updT_o", [H, QPAD], F32, isOutput=True)
    qidx_o = nc.declare_dram_parameter("qidx_o", [QPAD], U32, isOutput=True)

    wq_r = wq_d[:].rearrange("(k p) j -> p k j", p=128)
    wk_r = wkT_d[:].rearrange("(k p) j -> p k j", p=128)
    wv_r = wv_d[:].rearrange("(k p) j -> p k j", p=128)
    wo_r = wo_d[:].rearrange("(k p) j -> p k j", p=128)

    def emit_body(tc, rep):
        with (
            tc.tile_pool(name=f"glob{rep}", bufs=1) as glob,
            tc.tile_pool(name=f"wsl{rep}", bufs=2) as wslp,
            tc.tile_pool(name=f"ps{rep}", bufs=6, space="PSUM") as ps,
            tc.tile_pool(name=f"pst{rep}", bufs=2, space="PSUM") as pst,
            tc.tile_pool(name=f"dram{rep}", bufs=1, space="DRAM") as dpool,
        ):
            ident = glob.tile([128, 128], F32, tag="ident")
            make_identity(nc, ident[:])
            b_nat = glob.tile([128, NQCH, H], F32, tag="bnat")
            # btb8 shares its slot with aT (btb8 dead after screening)
            btb8 = glob.tile([128, KCH, QPAD], FP8, tag="btb_aT")
            cvals = [glob.tile([128, 512], F32, tag=f"cv{ci}", name=f"cv{ci}")
                     for ci in range(NQCH)]
            cidx = [glob.tile([128, 512], U32, tag=f"cxi{ci}", name=f"cxi{ci}")
                    for ci in range(NQCH)]
            wi1_s = glob.tile([128, KCH, 512], BF16, tag="wi1")
            wi2_s = glob.tile([128, 4], F32R, tag="wi2")
            wi2x_s = glob.tile([128, 4], F32, tag="wi2x")
            bi1_s = glob.tile([128, 4], F32, tag="bi1")
            bq32_s = glob.tile([128, 8], F32, tag="bq32")
            bv_s = glob.tile([128, 8], F32, tag="bvs")
            bo_s = glob.tile([128, 8], F32, tag="bos")
            cbase = glob.tile([128, 512], F32, tag="cbase")
            qlo = glob.tile([128, 4], F32, tag="qlo")
            qhi = glob.tile([128, 4], F32, tag="qhi")
            bstg = glob.tile([8, 1], F32, tag="bstg")
            qfg = glob.tile([8, 64], F32, tag="qfg")
            xqT = glob.tile([128, KCH, QPAD], F32, tag="xqT")
            # qpT dies after stage 5; its slot hosts wi1x (stage 2b).
            qpT = glob.tile([128, KCH, QPAD], F32, tag="qpT_wi1x")
            nc.sync.dma_start(wi1_s[:], wi1b_d[:].rearrange("(k p) j -> p k j", p=128))
            nc.sync.dma_start(wi2_s[:], wi2r_d[:])
            nc.sync.dma_start(wi2x_s[:], wi2s_d[:])
            nc.sync.dma_start(bi1_s[:], bi1s_d[:])
            nc.sync.dma_start(bq32_s[:], bq32s_d[:])
            nc.sync.dma_start(bv_s[:], bvs_d[:])
            nc.sync.dma_start(bo_s[:], bos_d[:])
            nc.sync.dma_start(cbase[:], cbase_d[:])
            nc.sync.dma_start(qlo[:], qlo_d[:])
            nc.sync.dma_start(qhi[:], qhi_d[:])
            nc.sync.dma_start(bstg[:], bstg_d[:])

            imp_dram = dpool.tile([TC], F32)
            qidx_dram = dpool.tile([QPAD], U32)

            # ============ Phase A: imp MLP, top-51, Xq, QpT, B ============
            with tc.tile_pool(name=f"pA{rep}", bufs=2) as pA, \
                 tc.tile_pool(name=f"pA1{rep}", bufs=1) as pA1:
                # ---- Stage 1: importance screen (bf16) over own 4096 ----
                xtc_r = xtcb_d[:].rearrange("(k p) t -> p k t", p=128)
                for nt in range(8):
                    xtile = pA.tile([128, KCH, 512], BF16, tag="xstream")
                    nc.sync.dma_start(
                        xtile[:], xtc_r[:, :, nt * 512:(nt + 1) * 512])
                    hT = pA.tile([128, 4, 512], F32R, tag="hT")
                    for mp in range(4):
                        hps = ps.tile([128, 512], F32, tag="pp")
                        for k in range(KCH):
                            nc.tensor.matmul(
                                out=hps[:],
                                lhsT=wi1_s[:, k, mp * 128:(mp + 1) * 128],
                                rhs=xtile[:, k, :],
                                start=(k == 0), stop=(k == KCH - 1),
                            )
                        nc.scalar.activation(
                            out=hT[:, mp, :], in_=hps[:], func=ACT.Relu,
                            bias=bi1_s[:, mp:mp + 1], scale=1.0,
                        )
                    ips = ps.tile([1, 512], F32, tag="pp")
                    for kp in range(4):
                        nc.tensor.matmul(
                            out=ips[:], lhsT=wi2_s[:, kp:kp + 1],
                            rhs=hT[:, kp, :],
                            start=(kp == 0), stop=(kp == 3),
                        )
                    impb = pA.tile([1, 512], F32, tag="impb")
                    nc.vector.tensor_copy(impb[:], ips[:])
                    nc.sync.dma_start(
                        imp_dram[nt * 512:(nt + 1) * 512].rearrange(
                            "(a b) -> a b", a=1),
                        impb[:],
                    )

                # ---- Stage 2: approx top-64 candidates per message ----
                imp8 = pA1.tile([8, 512], F32, tag="imp8")
                nc.sync.dma_start(
                    imp8[:], imp_dram[:].rearrange("(m t) -> m t", m=8))
                imp8w = pA1.tile([8, 512], F32, tag="imp8w")
                nc.vector.tensor_copy(imp8w[:], imp8[:])
                v64 = pA1.tile([8, 64], F32, tag="v64")
                i64 = pA1.tile([8, 64], U32, tag="i64")
                for r in range(8):
                    sl = slice(r * 8, r * 8 + 8)
                    nc.vector.max(out=v64[:, sl], in_=imp8w[:])
                    nc.vector.max_index(out=i64[:, sl], in_max=v64[:, sl],
                                        in_values=imp8[:])
                    if r < 7:
                        nc.vector.match_replace(
                            out=imp8w[:], in_to_replace=v64[:, sl],
                            in_values=imp8w[:], imm_value=NEG,
                        )
                # global query index (also kept in glob for the 2b export)
                nc.vector.tensor_copy(qfg[:], i64[:])       # u32 -> f32
                nc.vector.tensor_scalar(
                    out=qfg[:], in0=qfg[:], scalar1=bstg[:, :1],
                    scalar2=float(T - 1), op0=AOP.add, op1=AOP.min,
                )
                qu = pA1.tile([8, 64], U32, tag="qu")
                nc.vector.tensor_copy(qu[:], qfg[:])        # f32 -> u32
                nc.sync.dma_start(
                    qidx_dram[:].rearrange("(m t) -> m t", m=8), qu[:])

                # ---- Stage 3: gather Xq rows (exact f32),
                #      transpose to XqT (kept in F32 and F32R views) ----
                for ci in range(NQCH):
                    qt = pA.tile([128, 1], U32, tag="qix")
                    nc.sync.dma_start(
                        qt[:], qidx_dram[ci * 128:(ci + 1) * 128, None])
                    xq = pA.tile([128, H], F32, tag="xq")
                    nc.gpsimd.indirect_dma_start(
                        out=xq[:], out_offset=None, in_=x_d[:],
                        in_offset=bass.IndirectOffsetOnAxis(
                            ap=qt[:, :1], axis=0),
                    )
                    for k in range(KCH):
                        tps = pst.tile([128, 128], F32, tag="tp")
                        nc.tensor.transpose(
                            out=tps[:], in_=xq[:, k * 128:(k + 1) * 128],
                            identity=ident[:],
                        )
                        nc.vector.tensor_copy(
                            xqT[:, k, ci * 128:(ci + 1) * 128], tps[:])

                # ---- Stage 4: QpT[h',q] = (Wq^T XqT)/32 + bq/32 (exact
                #      fp32: f32r here adds ~8e-3 abs score noise, enough
                #      to flip top-5 membership at near-ties) ----
                for mp in range(KCH):
                    wsl = wslp.tile([128, KCH, 128], F32, tag="wsl")
                    nc.sync.dma_start(
                        wsl[:], wq_r[:, :, mp * 128:(mp + 1) * 128])
                    qps = ps.tile([128, QPAD], F32, tag="pp")
                    for k in range(KCH):
                        nc.tensor.matmul(
                            out=qps[:], lhsT=wsl[:, k, :], rhs=xqT[:, k, :],
                            start=(k == 0), stop=(k == KCH - 1),
                        )
                    nc.scalar.activation(
                        out=qpT[:, mp, :], in_=qps[:], func=ACT.Identity,
                        bias=bq32_s[:, mp:mp + 1], scale=1.0 / 32.0,
                    )

                # ---- Stage 5: B[q,h] = QpT^T @ WkT (f32r, natural) ----
                for hh in range(2):
                    wkh = pA1.tile([128, KCH, 512], F32, tag="wkh",
                                   name="wkh")
                    nc.sync.dma_start(
                        wkh[:], wk_r[:, :, hh * 512:(hh + 1) * 512])
                    for ci in range(NQCH):
                        bps = ps.tile([128, 512], F32, tag="pp")
                        for k in range(KCH):
                            nc.tensor.matmul(
                                out=bps[:],
                                lhsT=qpT[:, k, ci * 128:(ci + 1) * 128],
                                rhs=wkh[:, k, :],
                                start=(k == 0), stop=(k == KCH - 1),
                            )
                        nc.scalar.copy(
                            out=b_nat[:, ci, hh * 512:(hh + 1) * 512],
                            in_=bps[:])

                # btb8[h,q] (fp8) = B^T via PE transpose
                for ci in range(NQCH):
                    for k in range(KCH):
                        tps = pst.tile([128, 128], F32, tag="tp")
                        nc.tensor.transpose(
                            out=tps[:],
                            in_=b_nat[:, ci, k * 128:(k + 1) * 128],
                            identity=ident[:],
                        )
                        nc.vector.tensor_copy(
                            btb8[:, k, ci * 128:(ci + 1) * 128], tps[:])

            # ============ Phase B: fp8 DoubleRow screen + block top-8 ======
            xt8_r = xt8_d[:].rearrange("(k p) t -> p k t", p=128)
            if stages < 2:
                for ci in range(NQCH):
                    nc.vector.memset(cvals[ci][:], 0.0)
                    nc.vector.memset(cidx[ci][:], 0)
            with tc.tile_pool(name=f"pB{rep}", bufs=2) as pB:
                for tb2 in range(NTB // 2 if stages >= 2 else 0):
                    xt = pB.tile([128, KCH, 1024], FP8, tag="xt8s")
                    nc.sync.dma_start(
                        xt[:], xt8_r[:, :, tb2 * 1024:(tb2 + 1) * 1024])
                    for sb in range(2):
                        tb = tb2 * 2 + sb
                        sl5 = slice(sb * 512, (sb + 1) * 512)
                        for ci in range(NQCH):
                            sps = ps.tile([128, 512], F32, tag="pp")
                            for j in range(KCH // 2):
                                nc.tensor.matmul(
                                    out=sps[:],
                                    lhsT=btb8[:, 2 * j:2 * j + 2,
                                              ci * 128:(ci + 1) * 128],
                                    rhs=xt[:, 2 * j:2 * j + 2, sl5],
                                    start=(j == 0), stop=(j == KCH // 2 - 1),
                                    perf_mode=DR,
                                )
                            sl = slice(tb * 8, tb * 8 + 8)
                            nc.vector.max(out=cvals[ci][:, sl], in_=sps[:])
                            nc.vector.max_index(out=cidx[ci][:, sl],
                                                in_max=cvals[ci][:, sl],
                                                in_values=sps[:])

            # === Stage 2b PE part (off critical path): exact cand. imp ===
            # Emitted after B so its PE work fills the vector-bound phase C;
            # the DVE sort + sentinel export is emitted after phase C.
            # wi1x reuses qpT's slot (dead after stage 5).
            wi1x_s = glob.tile([128, KCH, 512], F32, tag="qpT_wi1x",
                               name="wi1xs")
            nc.sync.dma_start(
                wi1x_s[:], wi1x_d[:].rearrange("(k p) j -> p k j", p=128))
            hx = glob.tile([128, 4, QPAD], F32, tag="hx", name="hx")
            impx_dram = dpool.tile([QPAD], F32, name="impx_dram")
            for mp in range(4):
                hps2 = ps.tile([128, QPAD], F32, tag="pp", name="hps2")
                for k in range(KCH):
                    nc.tensor.matmul(
                        out=hps2[:],
                        lhsT=wi1x_s[:, k, mp * 128:(mp + 1) * 128],
                        rhs=xqT[:, k, :],
                        start=(k == 0), stop=(k == KCH - 1),
                    )
                nc.scalar.activation(
                    out=hx[:, mp, :], in_=hps2[:], func=ACT.Relu,
                    bias=bi1_s[:, mp:mp + 1], scale=1.0,
                )
            ipx = ps.tile([1, QPAD], F32, tag="pp", name="ipx")
            for kp in range(4):
                nc.tensor.matmul(
                    out=ipx[:], lhsT=wi2x_s[:, kp:kp + 1],
                    rhs=hx[:, kp, :], start=(kp == 0), stop=(kp == 3),
                )
            impxb = glob.tile([1, QPAD], F32, tag="impxb")
            nc.scalar.copy(out=impxb[:], in_=ipx[:])
            nc.sync.dma_start(
                impx_dram[:].rearrange("(a b) -> a b", a=1), impxb[:])

            # ============ Phase C: merge, exact rescue, attended ============
            aT = glob.tile([128, KCH, QPAD], F32R, tag="btb_aT")
            if stages < 3:
                nc.vector.memset(aT[:], 0.0)
            with tc.tile_pool(name=f"pC{rep}", bufs=2) as pC, \
                 tc.tile_pool(name=f"xg{rep}", bufs=1) as xgp:
                for ci in range(NQCH if stages >= 3 else 0):
                    gidx = pC.tile([128, 512], F32, tag="gidx")
                    nc.vector.tensor_copy(gidx[:], cidx[ci][:])   # u32->f32
                    nc.vector.tensor_tensor(out=gidx[:], in0=gidx[:],
                                            in1=cbase[:], op=AOP.add)
                    # mask own-message candidates
                    t0 = pC.tile([128, 512], F32, tag="t0")
                    nc.vector.tensor_scalar(
                        out=t0[:], in0=gidx[:], scalar1=qlo[:, ci:ci + 1],
                        scalar2=None, op0=AOP.is_ge)
                    t1 = pC.tile([128, 512], F32, tag="t1")
                    nc.vector.tensor_scalar(
                        out=t1[:], in0=gidx[:], scalar1=qhi[:, ci:ci + 1],
                        scalar2=None, op0=AOP.is_lt)
                    nc.vector.tensor_tensor(out=t0[:], in0=t0[:], in1=t1[:],
                                            op=AOP.mult)
                    cm = pC.tile([128, 512], F32, tag="cm")
                    nc.vector.scalar_tensor_tensor(
                        out=cm[:], in0=t0[:], scalar=NEG, in1=cvals[ci][:],
                        op0=AOP.mult, op1=AOP.add,
                    )
                    # composite key: quantized score (9 bits) * 32768 + gidx
                    cq = pC.tile([128, 512], F32, tag="cq")
                    nc.vector.tensor_scalar(
                        out=cq[:], in0=cm[:], scalar1=-8.0, scalar2=7.96875,
                        op0=AOP.max, op1=AOP.min)
                    nc.vector.tensor_scalar(
                        out=cq[:], in0=cq[:], scalar1=8.0, scalar2=32.0,
                        op0=AOP.add, op1=AOP.mult)
                    cqu = pC.tile([128, 512], U32, tag="cqu")
                    nc.vector.tensor_copy(cqu[:], cq[:])
                    nc.vector.tensor_copy(cq[:], cqu[:])
                    comp = pC.tile([128, 512], F32, tag="comp")
                    nc.vector.scalar_tensor_tensor(
                        out=comp[:], in0=cq[:], scalar=32768.0, in1=gidx[:],
                        op0=AOP.mult, op1=AOP.add,
                    )
                    # merged top-16 (fp8 screen noise ~0.05 + 1/32 quant can
                    # push a true-top-5 past rank 8 among the 512 candidates;
                    # measured 61/3264 at top-8, 0/3264 at top-16)
                    m16 = pC.tile([128, 16], F32, tag="m16")
                    nc.vector.max(out=m16[:, 0:8], in_=comp[:])
                    nc.vector.match_replace(
                        out=comp[:], in_to_replace=m16[:, 0:8],
                        in_values=comp[:], imm_value=NEG,
                    )
                    nc.vector.max(out=m16[:, 8:16], in_=comp[:])
                    # recover gidx = m16 mod 32768 (robust to round-vs-trunc)
                    dq = pC.tile([128, 16], F32, tag="dq")
                    nc.vector.tensor_scalar(
                        out=dq[:], in0=m16[:], scalar1=1.0 / 32768.0,
                        scalar2=None, op0=AOP.mult)
                    dqu = pC.tile([128, 16], U32, tag="dqu")
                    nc.vector.tensor_copy(dqu[:], dq[:])
                    nc.vector.tensor_copy(dq[:], dqu[:])
                    idx16f = pC.tile([128, 16], F32, tag="idx16f")
                    nc.vector.scalar_tensor_tensor(
                        out=idx16f[:], in0=dq[:], scalar=-32768.0, in1=m16[:],
                        op0=AOP.mult, op1=AOP.add,
                    )
                    neg = pC.tile([128, 16], F32, tag="negf")
                    nc.vector.tensor_scalar(
                        out=neg[:], in0=idx16f[:], scalar1=0.0, scalar2=32768.0,
                        op0=AOP.is_lt, op1=AOP.mult)
                    nc.vector.tensor_tensor(out=idx16f[:], in0=idx16f[:],
                                            in1=neg[:], op=AOP.add)
                    idx16u = pC.tile([128, 16], U32, tag="idx16u")
                    nc.vector.tensor_copy(idx16u[:], idx16f[:])

                    # gather candidate rows (f32) + exact fp32 rescore.
                    # 8 rows live in the xg pool; 8 more borrow the xqT and
                    # qpT_wi1x glob slots (dead once stage 2b's PE part ran).
                    xg = [xgp.tile([128, H], F32, tag=f"xg{r}", name=f"xg{r}")
                          for r in range(8)]
                    xgA = glob.tile([128, 4, H], F32, tag="xqT",
                                    name=f"xgA{ci}")
                    xgB = glob.tile([128, 4, H], F32, tag="qpT_wi1x",
                                    name=f"xgB{ci}")
                    rows = [t[:] for t in xg] + [xgA[:, j, :] for j in range(4)] \
                        + [xgB[:, j, :] for j in range(4)]
                    s16 = pC.tile([128, 16], F32, tag="s16")
                    for r in range(16):
                        if sub >= 1:
                            nc.gpsimd.indirect_dma_start(
                                out=rows[r], out_offset=None, in_=x_d[:],
                                in_offset=bass.IndirectOffsetOnAxis(
                                    ap=idx16u[:, r:r + 1], axis=0),
                            )
                        else:
                            nc.vector.memset(rows[r], 0.5)
                        if sub >= 2:
                            scr = pC.tile([128, H], F32, tag="scr")
                            nc.vector.scalar_tensor_tensor(
                                out=scr[:], in0=b_nat[:, ci, :], scalar=1.0,
                                in1=rows[r], op0=AOP.mult, op1=AOP.mult,
                                accum_out=s16[:, r:r + 1],
                            )
                    if sub < 2:
                        nc.vector.tensor_copy(s16[:], m16[:])

                    # exact top-5 softmax over the 16 rescored candidates
                    srt8 = pC.tile([128, 8], F32, tag="srt8")
                    nc.vector.max(out=srt8[:], in_=s16[:])
                    nmax = pC.tile([128, 1], F32, tag="nmax")
                    nc.vector.tensor_scalar(
                        out=nmax[:], in0=srt8[:, 0:1], scalar1=-1.0,
                        scalar2=None, op0=AOP.mult)
                    e16 = pC.tile([128, 16], F32, tag="e16")
                    nc.scalar.activation(out=e16[:], in_=s16[:], func=ACT.Exp,
                                         bias=nmax[:, :1], scale=1.0)
                    msk = pC.tile([128, 16], F32, tag="msk")
                    nc.vector.tensor_scalar(
                        out=msk[:], in0=s16[:], scalar1=srt8[:, 4:5],
                        scalar2=None, op0=AOP.is_ge)
                    zsum = pC.tile([128, 1], F32, tag="zsum")
                    nc.vector.scalar_tensor_tensor(
                        out=e16[:], in0=e16[:], scalar=1.0, in1=msk[:],
                        op0=AOP.mult, op1=AOP.mult, accum_out=zsum[:, :1],
                    )
                    rz = pC.tile([128, 1], F32, tag="rz")
                    if sub >= 3:
                        nc.vector.reciprocal(out=rz[:], in_=zsum[:])
                    else:
                        nc.vector.memset(rz[:], 0.2)
                    attn = pC.tile([128, 16], F32, tag="attn")
                    nc.vector.tensor_scalar(
                        out=attn[:], in0=e16[:], scalar1=rz[:, :1],
                        scalar2=None, op0=AOP.mult)

                    # attended = sum_r attn_r * row_r ; transpose into aT
                    acc = pC.tile([128, H], F32, tag="acc")
                    nc.vector.tensor_scalar(
                        out=acc[:], in0=rows[0], scalar1=attn[:, 0:1],
                        scalar2=None, op0=AOP.mult)
                    for r in range(1, 16):
                        nc.vector.scalar_tensor_tensor(
                            out=acc[:], in0=rows[r], scalar=attn[:, r:r + 1],
                            in1=acc[:], op0=AOP.mult, op1=AOP.add,
                        )
                    for k in range(KCH):
                        tps = pst.tile([128, 128], F32, tag="tp")
                        nc.tensor.transpose(
                            out=tps[:], in_=acc[:, k * 128:(k + 1) * 128],
                            identity=ident[:],
                        )
                        nc.vector.tensor_copy(
                            aT[:, k, ci * 128:(ci + 1) * 128], tps[:])

            # ====== Stage 2b sort + sentinel export (DVE, tail) ======
            with tc.tile_pool(name=f"p2b{rep}", bufs=1) as p2b:
                vx = p2b.tile([8, 64], F32, tag="vx")
                nc.sync.dma_start(
                    vx[:], impx_dram[:].rearrange("(m t) -> m t", m=8))
                vxw = p2b.tile([8, 64], F32, tag="vxw")
                nc.vector.tensor_copy(vxw[:], vx[:])
                v8r = p2b.tile([8, 8], F32, tag="v8r")
                for r in range(6):
                    nc.vector.max(out=v8r[:], in_=vxw[:])
                    nc.vector.match_replace(
                        out=vxw[:], in_to_replace=v8r[:],
                        in_values=vxw[:], imm_value=NEG,
                    )
                nc.vector.max(out=v8r[:], in_=vxw[:])   # ranks 49..56
                # theta = exact 51st (col 2); valid = exact imp >= theta
                vm = p2b.tile([8, 64], F32, tag="vm")
                nc.vector.tensor_scalar(
                    out=vm[:], in0=vx[:], scalar1=v8r[:, 2:3], scalar2=None,
                    op0=AOP.is_ge)
                qsent = p2b.tile([8, 64], F32, tag="qsent")
                nc.vector.tensor_tensor(out=qsent[:], in0=qfg[:], in1=vm[:],
                                        op=AOP.mult)
                sent2 = p2b.tile([8, 64], F32, tag="sent2")
                nc.vector.tensor_scalar(
                    out=sent2[:], in0=vm[:], scalar1=-1.0e9, scalar2=1.0e9,
                    op0=AOP.mult, op1=AOP.add)
                nc.vector.tensor_tensor(out=qsent[:], in0=qsent[:],
                                        in1=sent2[:], op=AOP.add)
                qsu = p2b.tile([8, 64], U32, tag="qsu")
                nc.vector.tensor_copy(qsu[:], qsent[:])
                nc.sync.dma_start(
                    qidx_o[:].rearrange("(m t) -> m t", m=8), qsu[:])

            # ============ Phase D: output projections (f32r) ============
            if stages < 4:
                with tc.tile_pool(name=f"pDz{rep}", bufs=1) as pDz:
                    z = pDz.tile([128, KCH, QPAD], F32, tag="z")
                    nc.vector.memset(z[:], 0.0)
                    nc.sync.dma_start(
                        updT_o[:].rearrange("(k p) q -> p k q", p=128), z[:])
            with tc.tile_pool(name=f"pD{rep}", bufs=1) as pD:
                vT = pD.tile([128, KCH, QPAD], F32R, tag="vT")
                for mp in range(KCH if stages >= 4 else 0):
                    wsl = wslp.tile([128, KCH, 128], F32R, tag="wslr", name="wslr")
                    nc.sync.dma_start(
                        wsl[:], wv_r[:, :, mp * 128:(mp + 1) * 128])
                    vps = ps.tile([128, QPAD], F32, tag="pp")
                    for k in range(KCH):
                        nc.tensor.matmul(
                            out=vps[:], lhsT=wsl[:, k, :], rhs=aT[:, k, :],
                            start=(k == 0), stop=(k == KCH - 1),
                        )
                    nc.scalar.activation(
                        out=vT[:, mp, :], in_=vps[:], func=ACT.Identity,
                        bias=bv_s[:, mp:mp + 1], scale=1.0,
                    )
                upd = pD.tile([128, KCH, QPAD], F32, tag="upd")
                for mp in range(KCH if stages >= 4 else 0):
                    wsl = wslp.tile([128, KCH, 128], F32R, tag="wslr", name="wslr")
                    nc.sync.dma_start(
                        wsl[:], wo_r[:, :, mp * 128:(mp + 1) * 128])
                    ups = ps.tile([128, QPAD], F32, tag="pp")
                    for k in range(KCH):
                        nc.tensor.matmul(
                            out=ups[:], lhsT=wsl[:, k, :], rhs=vT[:, k, :],
                            start=(k == 0), stop=(k == KCH - 1),
                        )
                    nc.scalar.activation(
                        out=upd[:, mp, :], in_=ups[:], func=ACT.Identity,
                        bias=bo_s[:, mp:mp + 1], scale=1.0,
                    )
                if stages >= 4:
                    nc.sync.dma_start(
                        updT_o[:].rearrange("(k p) q -> p k q", p=128), upd[:])

    with tile.TileContext(nc) as tc:
        for rep in range(reps):
            emit_body(tc, rep)

    nc.compile()
    return nc


def _host_prep(inputs):
    import ml_dtypes

    X = np.ascontiguousarray(np.asarray(inputs["token_features"],
                                        dtype=np.float32))
    B = np.asarray(inputs["message_boundaries"]).astype(np.int64)
    starts, ends = B[:, 0], B[:, 1]

    XT = np.ascontiguousarray(X.T)                     # [H, T] f32
    XT8 = np.clip(XT, -240.0, 240.0).astype(ml_dtypes.float8_e4m3fn)

    w = {k: np.ascontiguousarray(np.asarray(inputs[k], dtype=np.float32))
         for k in ("Wq", "Wk", "Wv", "Wo", "Wi1")}
    b = {k: np.asarray(inputs[k], dtype=np.float32)
         for k in ("bq", "bk", "bv", "bo", "bi1", "bi2")}
    Wi2 = np.asarray(inputs["Wi2"], dtype=np.float32)  # [512, 1]

    # tokens t with t2m[t]==m form [ends[m-1], ends[m])
    mlo = np.concatenate([[0], ends[:-1]]).astype(np.float32)
    mhi = ends.astype(np.float32)

    common = {
        "x": X,
        "xt8": XT8,
        "wq": w["Wq"],
        "wkT": np.ascontiguousarray(w["Wk"].T),
        "wv": w["Wv"],
        "wo": w["Wo"],
        "wi1b": w["Wi1"].astype(ml_dtypes.bfloat16),
        "wi1x": w["Wi1"],
        "wi2s": np.ascontiguousarray(Wi2[:, 0].reshape(4, 128).T),
        "wi2r": np.ascontiguousarray(Wi2[:, 0].reshape(4, 128).T),
        "bi1s": np.ascontiguousarray(b["bi1"].reshape(4, 128).T),
        "bq32s": np.ascontiguousarray((b["bq"] / 32.0).reshape(8, 128).T),
        "bvs": np.ascontiguousarray(b["bv"].reshape(8, 128).T),
        "bos": np.ascontiguousarray(b["bo"].reshape(8, 128).T),
        "cbase": np.ascontiguousarray(
            np.tile(((np.arange(512) // 8) * 512).astype(np.float32)[None, :],
                    (128, 1))),
    }

    in_maps = []
    for c in range(NCORES):
        msgs = np.arange(c * MSG_PER_CORE, (c + 1) * MSG_PER_CORE)
        row_m = np.repeat(msgs, QPAD // MSG_PER_CORE)   # [512], 64 per msg
        qlo_row = mlo[row_m].astype(np.float32)
        qhi_row = mhi[row_m].astype(np.float32)
        m = dict(common)
        m["xtcb"] = np.ascontiguousarray(
            XT[:, c * TC:(c + 1) * TC]).astype(ml_dtypes.bfloat16)
        m["bstg"] = starts[msgs].astype(np.float32).reshape(8, 1)
        m["qlo"] = np.ascontiguousarray(qlo_row.reshape(4, 128).T)
        m["qhi"] = np.ascontiguousarray(qhi_row.reshape(4, 128).T)
        in_maps.append(m)
    return in_maps


def _numpy_fallback(inputs):
    """Reference semantics in numpy (only for non-equal-length boundaries,
    which the stated harness never produces)."""
    X = np.asarray(inputs["token_features"], dtype=np.float32)
    B = np.asarray(inputs["message_boundaries"]).astype(np.int64)
    imp = (np.maximum(X @ np.asarray(inputs["Wi1"]) + np.asarray(inputs["bi1"]),
                      0) @ np.asarray(inputs["Wi2"])
           + np.asarray(inputs["bi2"]))[:, 0]
    k_imp = max(1, int((T // M) * 0.1))
    impm = imp.reshape(M, T // M)
    top_local = np.argsort(-impm, axis=1, kind="stable")[:, :k_imp]
    qidx_raw = (top_local + B[:, 0:1]).reshape(-1)
    qidx = np.minimum(qidx_raw, T - 1)
    Qp = X[qidx] @ np.asarray(inputs["Wq"]) + np.asarray(inputs["bq"])
    Km = X @ np.asarray(inputs["Wk"]) + np.asarray(inputs["bk"])
    S = (Qp @ Km.T) / np.float32(32.0)
    t2m = np.searchsorted(B[:, 1], np.arange(T), side="right")
    msg_ids = np.repeat(np.arange(M), k_imp)
    S[msg_ids[:, None] == t2m[None, :]] = -np.inf
    k_conn = min(5, T // M)
    top_idx = np.argsort(-S, axis=1, kind="stable")[:, :k_conn]
    tv = np.take_along_axis(S, top_idx, axis=1)
    e = np.exp(tv - tv[:, :1])
    attn = e / e.sum(1, keepdims=True)
    V = X @ np.asarray(inputs["Wv"]) + np.asarray(inputs["bv"])
    att = np.einsum("qk,qkh->qh", attn.astype(np.float32), V[top_idx])
    upd = att @ np.asarray(inputs["Wo"]) + np.asarray(inputs["bo"])
    out = X.copy()
    ok = qidx_raw < T
    np.add.at(out, qidx_raw[ok], upd[ok].astype(np.float32))
    return out


def kernel(**inputs):
    X = np.asarray(inputs["token_features"])
    B = np.asarray(inputs["message_boundaries"]).astype(np.int64)
    assert X.shape == (T, H), X.shape

    eq = (np.array_equal(B[:, 0], np.arange(M) * L)
          and np.array_equal(B[:, 1], (np.arange(M) + 1) * L))
    if not eq:
        return _numpy_fallback(inputs)

    from concourse.bass_utils import run_bass_kernel_spmd

    if "nc" not in _cache:
        _cache["nc"] = _build_program(
            int(os.environ.get("KERNEL_STAGES", "99")),
            int(os.environ.get("KERNEL_SUB", "9")),
            int(os.environ.get("KERNEL_REPS", "1")))
    nc = _cache["nc"]

    in_maps = _host_prep(inputs)
    trace = bool(int(os.environ.get("KERNEL_PROFILE", "0")))
    res = run_bass_kernel_spmd(nc, in_maps, list(range(NCORES)), trace=trace)
    if trace:
        _cache["exec_time_ns"] = res.exec_time_ns
        _cache["mean_exec_time_ns"] = res.mean_exec_time_ns

    out = np.array(X, dtype=np.float32, copy=True)
    idx_parts, upd_parts = [], []
    for c in range(NCORES):
        qi = res.results[c]["qidx_o"].astype(np.int64)
        valid = qi < T
        idx_parts.append(qi[valid])
        upd_parts.append(res.results[c]["updT_o"].T[valid].astype(np.float32))
    all_idx = np.concatenate(idx_parts)
    all_upd = np.concatenate(upd_parts)
    if len(np.unique(all_idx)) == len(all_idx):
        out[all_idx] += all_upd
    else:
        np.add.at(out, all_idx, all_upd)
    return out


# revision 32
# speedup vs baseline: 1.0667x; 1.0667x over previous
"""CrossMessageTokenAttention Trainium2 kernel (8 NeuronCores, SPMD).

Contract: kernel(**inputs) takes the FULL inputs of reference.setup_inputs()
and returns the FULL [32768, 1024] float32 output.

Strategy (hardcoded for T=32768, H=1024, M=64 messages, L=512, k_imp=51,
k_conn=5, 8 cores):
  - Queries (64 msgs x 51 selected tokens) are sharded by message: core c
    owns messages [8c, 8c+8) = 408 queries (padded to 512). No collectives.
  - Algebra: scores = ((Xq@Wq+bq)/32) @ Wk^T @ X^T  (bk dropped: a per-row
    shift, invariant for top-k and softmax). Values never materialized:
    attended = (sum_k attn_k X[idx_k]) @ Wv + bv, then @ Wo + bo.
  - Importance screen runs in bf16 (error ~1e-2 sigma vs a ~1.5e-1 sigma
    top-64 margin); the exact top-51 threshold is rescued in true fp32 on
    the 512 gathered candidate rows (stage 2b). 2b's PE part is emitted
    after phase B and its DVE sort after phase C, so both hide under
    phases that don't use those engines.
  - B = Xq @ (Wq Wk^T/32) in ONE exact-fp32 matmul stage; W_qk is
    precomputed on the host in f64 (valid since bq == 0, guarded).
    f32r here would add ~8e-3 abs score noise and flip top-5 membership
    at near-ties (~100 rows); fp32 is required.
  - The dominant [512,1024]x[1024,32768] score matmul runs in fp8e4
    DoubleRow (2 contraction rows/cycle, 2x bf16) as a SCREEN: per
    512-token block, HW top-8 (vector.max/max_index) read from PSUM.
  - Same-message masking on candidates by global-index range compare (each
    message is one aligned 512-token block).
  - Merge of the 512 candidates: quantized-score composite key
    q*32768+gidx (exact in f32 ints <= 2^24), two max8 passes -> top-16;
    index recovered by residue (robust to round-vs-trunc casts). Top-16
    (not 8): fp8 screen noise ~0.05 + 1/32 quantization push a true
    top-5 past rank 8 for ~2% of queries (measured 61/3264); at 16 it is
    0/3264.
  - Exact rescue: gather the 16 candidate rows in f32 (bf16 rows would add
    ~0.1 abs score noise, comparable to top-5 gaps), recompute their
    scores in fp32 on the vector engine against B, rank + softmax on the
    rescored values, weighted-sum the gathered rows.
  - Device outputs per core: updates^T [1024, 512] and query indices [512]
    (408 valid). The host scatters updates into a copy of token_features.
"""
import os

os.environ.setdefault("MYCRO_LOCAL_CACHE", "1")

import numpy as np

T, H = 32768, 1024
M, L = 64, 512
K_IMP = 51
NCORES = 8
MSG_PER_CORE = M // NCORES          # 8
QC = MSG_PER_CORE * K_IMP           # 408 queries per core
QPAD = 512
NQCH = QPAD // 128                  # 4
KCH = H // 128                      # 8
NTB = T // 512                      # 64 score blocks
TC = T // NCORES                    # 4096 tokens per core block
NEG = -1.0e30

_cache = {}


def _build_program(stages=99, sub=9, reps=1):
    import concourse.bacc as bacc
    import concourse.bass as bass
    import concourse.mybir as mybir
    import concourse.tile as tile
    from concourse.masks import make_identity

    F32 = mybir.dt.float32
    F32R = mybir.dt.float32r
    BF16 = mybir.dt.bfloat16
    FP8 = mybir.dt.float8e4
    U32 = mybir.dt.uint32
    AOP = mybir.AluOpType
    ACT = mybir.ActivationFunctionType
    AXX = mybir.AxisListType.X
    DR = mybir.MatmulPerfMode.DoubleRow

    nc = bacc.Bacc(None, target_bir_lowering=False, debug=False,
                   num_devices=NCORES)

    # ---------------- DRAM I/O ----------------
    x_d = nc.declare_dram_parameter("x", [T, H], F32, isOutput=False)
    xt8_d = nc.declare_dram_parameter("xt8", [H, T], FP8, isOutput=False)
    xtcb_d = nc.declare_dram_parameter("xtcb", [H, TC], BF16, isOutput=False)
    wqk_d = nc.declare_dram_parameter("wqk", [H, H], F32, isOutput=False)
    wv_d = nc.declare_dram_parameter("wv", [H, H], F32R, isOutput=False)
    wo_d = nc.declare_dram_parameter("wo", [H, H], F32R, isOutput=False)
    wi1b_d = nc.declare_dram_parameter("wi1b", [H, H // 2], BF16, isOutput=False)
    wi1x_d = nc.declare_dram_parameter("wi1x", [H, H // 2], F32, isOutput=False)
    wi2r_d = nc.declare_dram_parameter("wi2r", [128, 4], F32R, isOutput=False)
    wi2s_d = nc.declare_dram_parameter("wi2s", [128, 4], F32, isOutput=False)
    bi1s_d = nc.declare_dram_parameter("bi1s", [128, 4], F32, isOutput=False)
    bvs_d = nc.declare_dram_parameter("bvs", [128, 8], F32, isOutput=False)
    bos_d = nc.declare_dram_parameter("bos", [128, 8], F32, isOutput=False)
    bstg_d = nc.declare_dram_parameter("bstg", [8, 1], F32, isOutput=False)
    qlo_d = nc.declare_dram_parameter("qlo", [128, 4], F32, isOutput=False)
    qhi_d = nc.declare_dram_parameter("qhi", [128, 4], F32, isOutput=False)
    cbase_d = nc.declare_dram_parameter("cbase", [128, 512], F32, isOutput=False)

    updT_o = nc.declare_dram_parameter("updT_o", [H, QPAD], F32, isOutput=True)
    qidx_o = nc.declare_dram_parameter("qidx_o", [QPAD], U32, isOutput=True)

    wqk_r = wqk_d[:].rearrange("(k p) j -> p k j", p=128)
    wv_r = wv_d[:].rearrange("(k p) j -> p k j", p=128)
    wo_r = wo_d[:].rearrange("(k p) j -> p k j", p=128)

    def emit_body(tc, rep):
        with (
            tc.tile_pool(name=f"glob{rep}", bufs=1) as glob,
            tc.tile_pool(name=f"wsl{rep}", bufs=2) as wslp,
            tc.tile_pool(name=f"ps{rep}", bufs=6, space="PSUM") as ps,
            tc.tile_pool(name=f"pst{rep}", bufs=2, space="PSUM") as pst,
            tc.tile_pool(name=f"dram{rep}", bufs=1, space="DRAM") as dpool,
        ):
            ident = glob.tile([128, 128], F32, tag="ident")
            make_identity(nc, ident[:])
            b_nat = glob.tile([128, NQCH, H], F32, tag="bnat")
            # btb8 shares its slot with aT (btb8 dead after screening)
            btb8 = glob.tile([128, KCH, QPAD], FP8, tag="btb_aT")
            cvals = [glob.tile([128, 512], F32, tag=f"cv{ci}", name=f"cv{ci}")
                     for ci in range(NQCH)]
            cidx = [glob.tile([128, 512], U32, tag=f"cxi{ci}", name=f"cxi{ci}")
                    for ci in range(NQCH)]
            wi1_s = glob.tile([128, KCH, 512], BF16, tag="wi1")
            wi2_s = glob.tile([128, 4], F32R, tag="wi2")
            wi2x_s = glob.tile([128, 4], F32, tag="wi2x")
            bi1_s = glob.tile([128, 4], F32, tag="bi1")
            bv_s = glob.tile([128, 8], F32, tag="bvs")
            bo_s = glob.tile([128, 8], F32, tag="bos")
            cbase = glob.tile([128, 512], F32, tag="cbase")
            qlo = glob.tile([128, 4], F32, tag="qlo")
            qhi = glob.tile([128, 4], F32, tag="qhi")
            bstg = glob.tile([8, 1], F32, tag="bstg")
            qfg = glob.tile([8, 64], F32, tag="qfg")
            xqT = glob.tile([128, KCH, QPAD], F32, tag="xqT")
            nc.sync.dma_start(wi1_s[:], wi1b_d[:].rearrange("(k p) j -> p k j", p=128))
            nc.sync.dma_start(wi2_s[:], wi2r_d[:])
            nc.sync.dma_start(wi2x_s[:], wi2s_d[:])
            nc.sync.dma_start(bi1_s[:], bi1s_d[:])
            nc.sync.dma_start(bv_s[:], bvs_d[:])
            nc.sync.dma_start(bo_s[:], bos_d[:])
            nc.sync.dma_start(cbase[:], cbase_d[:])
            nc.sync.dma_start(qlo[:], qlo_d[:])
            nc.sync.dma_start(qhi[:], qhi_d[:])
            nc.sync.dma_start(bstg[:], bstg_d[:])

            imp_dram = dpool.tile([TC], F32)
            qidx_dram = dpool.tile([QPAD], U32)

            # ============ Phase A: imp MLP, top-51, Xq, QpT, B ============
            with tc.tile_pool(name=f"pA{rep}", bufs=2) as pA, \
                 tc.tile_pool(name=f"pA1{rep}", bufs=1) as pA1:
                # ---- Stage 1: importance screen (bf16) over own 4096 ----
                xtc_r = xtcb_d[:].rearrange("(k p) t -> p k t", p=128)
                for nt in range(8):
                    xtile = pA.tile([128, KCH, 512], BF16, tag="xstream")
                    nc.sync.dma_start(
                        xtile[:], xtc_r[:, :, nt * 512:(nt + 1) * 512])
                    hT = pA.tile([128, 4, 512], F32R, tag="hT")
                    for mp in range(4):
                        hps = ps.tile([128, 512], F32, tag="pp")
                        for k in range(KCH):
                            nc.tensor.matmul(
                                out=hps[:],
                                lhsT=wi1_s[:, k, mp * 128:(mp + 1) * 128],
                                rhs=xtile[:, k, :],
                                start=(k == 0), stop=(k == KCH - 1),
                            )
                        nc.scalar.activation(
                            out=hT[:, mp, :], in_=hps[:], func=ACT.Relu,
                            bias=bi1_s[:, mp:mp + 1], scale=1.0,
                        )
                    ips = ps.tile([1, 512], F32, tag="pp")
                    for kp in range(4):
                        nc.tensor.matmul(
                            out=ips[:], lhsT=wi2_s[:, kp:kp + 1],
                            rhs=hT[:, kp, :],
                            start=(kp == 0), stop=(kp == 3),
                        )
                    impb = pA.tile([1, 512], F32, tag="impb")
                    nc.vector.tensor_copy(impb[:], ips[:])
                    nc.sync.dma_start(
                        imp_dram[nt * 512:(nt + 1) * 512].rearrange(
                            "(a b) -> a b", a=1),
                        impb[:],
                    )

                # ---- Stage 2: approx top-64 candidates per message ----
                imp8 = pA1.tile([8, 512], F32, tag="imp8")
                nc.sync.dma_start(
                    imp8[:], imp_dram[:].rearrange("(m t) -> m t", m=8))
                imp8w = pA1.tile([8, 512], F32, tag="imp8w")
                nc.vector.tensor_copy(imp8w[:], imp8[:])
                v64 = pA1.tile([8, 64], F32, tag="v64")
                i64 = pA1.tile([8, 64], U32, tag="i64")
                for r in range(8):
                    sl = slice(r * 8, r * 8 + 8)
                    nc.vector.max(out=v64[:, sl], in_=imp8w[:])
                    nc.vector.max_index(out=i64[:, sl], in_max=v64[:, sl],
                                        in_values=imp8[:])
                    if r < 7:
                        nc.vector.match_replace(
                            out=imp8w[:], in_to_replace=v64[:, sl],
                            in_values=imp8w[:], imm_value=NEG,
                        )
                # global query index (also kept in glob for the 2b export)
                nc.vector.tensor_copy(qfg[:], i64[:])       # u32 -> f32
                nc.vector.tensor_scalar(
                    out=qfg[:], in0=qfg[:], scalar1=bstg[:, :1],
                    scalar2=float(T - 1), op0=AOP.add, op1=AOP.min,
                )
                qu = pA1.tile([8, 64], U32, tag="qu")
                nc.vector.tensor_copy(qu[:], qfg[:])        # f32 -> u32
                nc.sync.dma_start(
                    qidx_dram[:].rearrange("(m t) -> m t", m=8), qu[:])

                # ---- Stage 3: gather Xq rows (exact f32),
                #      transpose to XqT (kept in F32 and F32R views) ----
                for ci in range(NQCH):
                    qt = pA.tile([128, 1], U32, tag="qix")
                    nc.sync.dma_start(
                        qt[:], qidx_dram[ci * 128:(ci + 1) * 128, None])
                    xq = pA.tile([128, H], F32, tag="xq")
                    nc.gpsimd.indirect_dma_start(
                        out=xq[:], out_offset=None, in_=x_d[:],
                        in_offset=bass.IndirectOffsetOnAxis(
                            ap=qt[:, :1], axis=0),
                    )
                    for k in range(KCH):
                        tps = pst.tile([128, 128], F32, tag="tp")
                        nc.tensor.transpose(
                            out=tps[:], in_=xq[:, k * 128:(k + 1) * 128],
                            identity=ident[:],
                        )
                        nc.vector.tensor_copy(
                            xqT[:, k, ci * 128:(ci + 1) * 128], tps[:])

                # ---- Stage 4+5 fused: B[q,h] = Xq @ (Wq WkT / 32)
                #      (W_qk precomputed on host in f64; exact-fp32 matmul.
                #      Valid because bq == 0 -- guarded in kernel()) ----
                for hh in range(2):
                    wkh = pA1.tile([128, KCH, 512], F32, tag="wkh",
                                   name="wkh")
                    nc.sync.dma_start(
                        wkh[:], wqk_r[:, :, hh * 512:(hh + 1) * 512])
                    for ci in range(NQCH):
                        bps = ps.tile([128, 512], F32, tag="pp")
                        for k in range(KCH):
                            nc.tensor.matmul(
                                out=bps[:],
                                lhsT=xqT[:, k, ci * 128:(ci + 1) * 128],
                                rhs=wkh[:, k, :],
                                start=(k == 0), stop=(k == KCH - 1),
                            )
                        nc.scalar.copy(
                            out=b_nat[:, ci, hh * 512:(hh + 1) * 512],
                            in_=bps[:])

                # btb8[h,q] (fp8) = B^T via PE transpose
                for ci in range(NQCH):
                    for k in range(KCH):
                        tps = pst.tile([128, 128], F32, tag="tp")
                        nc.tensor.transpose(
                            out=tps[:],
                            in_=b_nat[:, ci, k * 128:(k + 1) * 128],
                            identity=ident[:],
                        )
                        nc.vector.tensor_copy(
                            btb8[:, k, ci * 128:(ci + 1) * 128], tps[:])

            # ============ Phase B: fp8 DoubleRow screen + block top-8 ======
            xt8_r = xt8_d[:].rearrange("(k p) t -> p k t", p=128)
            if stages < 2:
                for ci in range(NQCH):
                    nc.vector.memset(cvals[ci][:], 0.0)
                    nc.vector.memset(cidx[ci][:], 0)
            with tc.tile_pool(name=f"pB{rep}", bufs=2) as pB:
                for tb2 in range(NTB // 2 if stages >= 2 else 0):
                    xt = pB.tile([128, KCH, 1024], FP8, tag="xt8s")
                    nc.sync.dma_start(
                        xt[:], xt8_r[:, :, tb2 * 1024:(tb2 + 1) * 1024])
                    for sb in range(2):
                        tb = tb2 * 2 + sb
                        sl5 = slice(sb * 512, (sb + 1) * 512)
                        for ci in range(NQCH):
                            sps = ps.tile([128, 512], F32, tag="pp")
                            for j in range(KCH // 2):
                                nc.tensor.matmul(
                                    out=sps[:],
                                    lhsT=btb8[:, 2 * j:2 * j + 2,
                                              ci * 128:(ci + 1) * 128],
                                    rhs=xt[:, 2 * j:2 * j + 2, sl5],
                                    start=(j == 0), stop=(j == KCH // 2 - 1),
                                    perf_mode=DR,
                                )
                            sl = slice(tb * 8, tb * 8 + 8)
                            nc.vector.max(out=cvals[ci][:, sl], in_=sps[:])
                            nc.vector.max_index(out=cidx[ci][:, sl],
                                                in_max=cvals[ci][:, sl],
                                                in_values=sps[:])

            # === Stage 2b PE part (off critical path): exact cand. imp ===
            # Emitted after B so its PE work fills the vector-bound phase C;
            # the DVE sort + sentinel export is emitted after phase C.
            wi1x_s = glob.tile([128, KCH, 512], F32, tag="wi1x_xgB",
                               name="wi1xs")
            nc.sync.dma_start(
                wi1x_s[:], wi1x_d[:].rearrange("(k p) j -> p k j", p=128))
            hx = glob.tile([128, 4, QPAD], F32, tag="hx", name="hx")
            impx_dram = dpool.tile([QPAD], F32, name="impx_dram")
            for mp in range(4):
                hps2 = ps.tile([128, QPAD], F32, tag="pp", name="hps2")
                for k in range(KCH):
                    nc.tensor.matmul(
                        out=hps2[:],
                        lhsT=wi1x_s[:, k, mp * 128:(mp + 1) * 128],
                        rhs=xqT[:, k, :],
                        start=(k == 0), stop=(k == KCH - 1),
                    )
                nc.scalar.activation(
                    out=hx[:, mp, :], in_=hps2[:], func=ACT.Relu,
                    bias=bi1_s[:, mp:mp + 1], scale=1.0,
                )
            ipx = ps.tile([1, QPAD], F32, tag="pp", name="ipx")
            for kp in range(4):
                nc.tensor.matmul(
                    out=ipx[:], lhsT=wi2x_s[:, kp:kp + 1],
                    rhs=hx[:, kp, :], start=(kp == 0), stop=(kp == 3),
                )
            impxb = glob.tile([1, QPAD], F32, tag="impxb")
            nc.scalar.copy(out=impxb[:], in_=ipx[:])
            nc.sync.dma_start(
                impx_dram[:].rearrange("(a b) -> a b", a=1), impxb[:])

            # ============ Phase C: merge, exact rescue, attended ============
            aT = glob.tile([128, KCH, QPAD], F32R, tag="btb_aT")
            if stages < 3:
                nc.vector.memset(aT[:], 0.0)
            with tc.tile_pool(name=f"pC{rep}", bufs=2) as pC, \
                 tc.tile_pool(name=f"xg{rep}", bufs=1) as xgp:
                for ci in range(NQCH if stages >= 3 else 0):
                    gidx = pC.tile([128, 512], F32, tag="gidx")
                    nc.vector.tensor_copy(gidx[:], cidx[ci][:])   # u32->f32
                    nc.vector.tensor_tensor(out=gidx[:], in0=gidx[:],
                                            in1=cbase[:], op=AOP.add)
                    # mask own-message candidates
                    t0 = pC.tile([128, 512], F32, tag="t0")
                    nc.vector.tensor_scalar(
                        out=t0[:], in0=gidx[:], scalar1=qlo[:, ci:ci + 1],
                        scalar2=None, op0=AOP.is_ge)
                    t1 = pC.tile([128, 512], F32, tag="t1")
                    nc.vector.tensor_scalar(
                        out=t1[:], in0=gidx[:], scalar1=qhi[:, ci:ci + 1],
                        scalar2=None, op0=AOP.is_lt)
                    nc.vector.tensor_tensor(out=t0[:], in0=t0[:], in1=t1[:],
                                            op=AOP.mult)
                    cm = pC.tile([128, 512], F32, tag="cm")
                    nc.vector.scalar_tensor_tensor(
                        out=cm[:], in0=t0[:], scalar=NEG, in1=cvals[ci][:],
                        op0=AOP.mult, op1=AOP.add,
                    )
                    # composite key: quantized score (9 bits) * 32768 + gidx
                    cq = pC.tile([128, 512], F32, tag="cq")
                    nc.vector.tensor_scalar(
                        out=cq[:], in0=cm[:], scalar1=-8.0, scalar2=7.96875,
                        op0=AOP.max, op1=AOP.min)
                    nc.vector.tensor_scalar(
                        out=cq[:], in0=cq[:], scalar1=8.0, scalar2=32.0,
                        op0=AOP.add, op1=AOP.mult)
                    cqu = pC.tile([128, 512], U32, tag="cqu")
                    nc.vector.tensor_copy(cqu[:], cq[:])
                    nc.vector.tensor_copy(cq[:], cqu[:])
                    comp = pC.tile([128, 512], F32, tag="comp")
                    nc.vector.scalar_tensor_tensor(
                        out=comp[:], in0=cq[:], scalar=32768.0, in1=gidx[:],
                        op0=AOP.mult, op1=AOP.add,
                    )
                    # merged top-16 (fp8 screen noise ~0.05 + 1/32 quant can
                    # push a true-top-5 past rank 8 among the 512 candidates;
                    # measured 61/3264 at top-8, 0/3264 at top-16)
                    m16 = pC.tile([128, 16], F32, tag="m16")
                    nc.vector.max(out=m16[:, 0:8], in_=comp[:])
                    nc.vector.match_replace(
                        out=comp[:], in_to_replace=m16[:, 0:8],
                        in_values=comp[:], imm_value=NEG,
                    )
                    nc.vector.max(out=m16[:, 8:16], in_=comp[:])
                    # recover gidx = m16 mod 32768 (robust to round-vs-trunc)
                    dq = pC.tile([128, 16], F32, tag="dq")
                    nc.vector.tensor_scalar(
                        out=dq[:], in0=m16[:], scalar1=1.0 / 32768.0,
                        scalar2=None, op0=AOP.mult)
                    dqu = pC.tile([128, 16], U32, tag="dqu")
                    nc.vector.tensor_copy(dqu[:], dq[:])
                    nc.vector.tensor_copy(dq[:], dqu[:])
                    idx16f = pC.tile([128, 16], F32, tag="idx16f")
                    nc.vector.scalar_tensor_tensor(
                        out=idx16f[:], in0=dq[:], scalar=-32768.0, in1=m16[:],
                        op0=AOP.mult, op1=AOP.add,
                    )
                    neg = pC.tile([128, 16], F32, tag="negf")
                    nc.vector.tensor_scalar(
                        out=neg[:], in0=idx16f[:], scalar1=0.0, scalar2=32768.0,
                        op0=AOP.is_lt, op1=AOP.mult)
                    nc.vector.tensor_tensor(out=idx16f[:], in0=idx16f[:],
                                            in1=neg[:], op=AOP.add)
                    idx16u = pC.tile([128, 16], U32, tag="idx16u")
                    nc.vector.tensor_copy(idx16u[:], idx16f[:])

                    # gather candidate rows (f32, one indirect DMA per
                    # row -- multi-row offset APs abort on device) + exact
                    # fp32 rescore. Two 4-row groups live in the xg pool;
                    # two borrow the xqT and wi1x glob slots (dead once
                    # stage 2b's PE part ran).
                    xgg = [xgp.tile([128, 4, H], F32, tag=f"xgg{g}",
                                    name=f"xgg{g}") for g in range(2)]
                    xgg.append(glob.tile([128, 4, H], F32, tag="xqT",
                                         name=f"xgA{ci}"))
                    xgg.append(glob.tile([128, 4, H], F32, tag="wi1x_xgB",
                                         name=f"xgB{ci}"))
                    rows = [xgg[r // 4][:, r % 4, :] for r in range(16)]
                    s16 = pC.tile([128, 16], F32, tag="s16")
                    for r in range(16):
                        if sub >= 1:
                            nc.gpsimd.indirect_dma_start(
                                out=rows[r], out_offset=None, in_=x_d[:],
                                in_offset=bass.IndirectOffsetOnAxis(
                                    ap=idx16u[:, r:r + 1], axis=0),
                            )
                        else:
                            nc.vector.memset(rows[r], 0.5)
                        if sub >= 2:
                            scr = pC.tile([128, H], F32, tag="scr")
                            nc.vector.scalar_tensor_tensor(
                                out=scr[:], in0=b_nat[:, ci, :], scalar=1.0,
                                in1=rows[r], op0=AOP.mult, op1=AOP.mult,
                                accum_out=s16[:, r:r + 1],
                            )
                    if sub < 2:
                        nc.vector.tensor_copy(s16[:], m16[:])

                    # exact top-5 softmax over the 16 rescored candidates
                    srt8 = pC.tile([128, 8], F32, tag="srt8")
                    nc.vector.max(out=srt8[:], in_=s16[:])
                    nmax = pC.tile([128, 1], F32, tag="nmax")
                    nc.vector.tensor_scalar(
                        out=nmax[:], in0=srt8[:, 0:1], scalar1=-1.0,
                        scalar2=None, op0=AOP.mult)
                    e16 = pC.tile([128, 16], F32, tag="e16")
                    nc.scalar.activation(out=e16[:], in_=s16[:], func=ACT.Exp,
                                         bias=nmax[:, :1], scale=1.0)
                    msk = pC.tile([128, 16], F32, tag="msk")
                    nc.vector.tensor_scalar(
                        out=msk[:], in0=s16[:], scalar1=srt8[:, 4:5],
                        scalar2=None, op0=AOP.is_ge)
                    zsum = pC.tile([128, 1], F32, tag="zsum")
                    nc.vector.scalar_tensor_tensor(
                        out=e16[:], in0=e16[:], scalar=1.0, in1=msk[:],
                        op0=AOP.mult, op1=AOP.mult, accum_out=zsum[:, :1],
                    )
                    rz = pC.tile([128, 1], F32, tag="rz")
                    if sub >= 3:
                        nc.vector.reciprocal(out=rz[:], in_=zsum[:])
                    else:
                        nc.vector.memset(rz[:], 0.2)
                    attn = pC.tile([128, 16], F32, tag="attn")
                    nc.vector.tensor_scalar(
                        out=attn[:], in0=e16[:], scalar1=rz[:, :1],
                        scalar2=None, op0=AOP.mult)

                    # attended = sum_r attn_r * row_r ; transpose into aT
                    acc = pC.tile([128, H], F32, tag="acc")
                    nc.vector.tensor_scalar(
                        out=acc[:], in0=rows[0], scalar1=attn[:, 0:1],
                        scalar2=None, op0=AOP.mult)
                    for r in range(1, 16):
                        nc.vector.scalar_tensor_tensor(
                            out=acc[:], in0=rows[r], scalar=attn[:, r:r + 1],
                            in1=acc[:], op0=AOP.mult, op1=AOP.add,
                        )
                    for k in range(KCH):
                        tps = pst.tile([128, 128], F32, tag="tp")
                        nc.tensor.transpose(
                            out=tps[:], in_=acc[:, k * 128:(k + 1) * 128],
                            identity=ident[:],
                        )
                        nc.vector.tensor_copy(
                            aT[:, k, ci * 128:(ci + 1) * 128], tps[:])

            # ====== Stage 2b sort + sentinel export (DVE, tail) ======
            with tc.tile_pool(name=f"p2b{rep}", bufs=1) as p2b:
                vx = p2b.tile([8, 64], F32, tag="vx")
                nc.sync.dma_start(
                    vx[:], impx_dram[:].rearrange("(m t) -> m t", m=8))
                vxw = p2b.tile([8, 64], F32, tag="vxw")
                nc.vector.tensor_copy(vxw[:], vx[:])
                v8r = p2b.tile([8, 8], F32, tag="v8r")
                for r in range(6):
                    nc.vector.max(out=v8r[:], in_=vxw[:])
                    nc.vector.match_replace(
                        out=vxw[:], in_to_replace=v8r[:],
                        in_values=vxw[:], imm_value=NEG,
                    )
                nc.vector.max(out=v8r[:], in_=vxw[:])   # ranks 49..56
                # theta = exact 51st (col 2); valid = exact imp >= theta
                vm = p2b.tile([8, 64], F32, tag="vm")
                nc.vector.tensor_scalar(
                    out=vm[:], in0=vx[:], scalar1=v8r[:, 2:3], scalar2=None,
                    op0=AOP.is_ge)
                qsent = p2b.tile([8, 64], F32, tag="qsent")
                nc.vector.tensor_tensor(out=qsent[:], in0=qfg[:], in1=vm[:],
                                        op=AOP.mult)
                sent2 = p2b.tile([8, 64], F32, tag="sent2")
                nc.vector.tensor_scalar(
                    out=sent2[:], in0=vm[:], scalar1=-1.0e9, scalar2=1.0e9,
                    op0=AOP.mult, op1=AOP.add)
                nc.vector.tensor_tensor(out=qsent[:], in0=qsent[:],
                                        in1=sent2[:], op=AOP.add)
                qsu = p2b.tile([8, 64], U32, tag="qsu")
                nc.vector.tensor_copy(qsu[:], qsent[:])
                nc.sync.dma_start(
                    qidx_o[:].rearrange("(m t) -> m t", m=8), qsu[:])

            # ============ Phase D: output projections (f32r) ============
            if stages < 4:
                with tc.tile_pool(name=f"pDz{rep}", bufs=1) as pDz:
                    z = pDz.tile([128, KCH, QPAD], F32, tag="z")
                    nc.vector.memset(z[:], 0.0)
                    nc.sync.dma_start(
                        updT_o[:].rearrange("(k p) q -> p k q", p=128), z[:])
            with tc.tile_pool(name=f"pD{rep}", bufs=1) as pD:
                vT = pD.tile([128, KCH, QPAD], F32R, tag="vT")
                for mp in range(KCH if stages >= 4 else 0):
                    wsl = wslp.tile([128, KCH, 128], F32R, tag="wslr", name="wslr")
                    nc.sync.dma_start(
                        wsl[:], wv_r[:, :, mp * 128:(mp + 1) * 128])
                    vps = ps.tile([128, QPAD], F32, tag="pp")
                    for k in range(KCH):
                        nc.tensor.matmul(
                            out=vps[:], lhsT=wsl[:, k, :], rhs=aT[:, k, :],
                            start=(k == 0), stop=(k == KCH - 1),
                        )
                    nc.scalar.activation(
                        out=vT[:, mp, :], in_=vps[:], func=ACT.Identity,
                        bias=bv_s[:, mp:mp + 1], scale=1.0,
                    )
                upd = pD.tile([128, KCH, QPAD], F32, tag="upd")
                for mp in range(KCH if stages >= 4 else 0):
                    wsl = wslp.tile([128, KCH, 128], F32R, tag="wslr", name="wslr")
                    nc.sync.dma_start(
                        wsl[:], wo_r[:, :, mp * 128:(mp + 1) * 128])
                    ups = ps.tile([128, QPAD], F32, tag="pp")
                    for k in range(KCH):
                        nc.tensor.matmul(
                            out=ups[:], lhsT=wsl[:, k, :], rhs=vT[:, k, :],
                            start=(k == 0), stop=(k == KCH - 1),
                        )
                    nc.scalar.activation(
                        out=upd[:, mp, :], in_=ups[:], func=ACT.Identity,
                        bias=bo_s[:, mp:mp + 1], scale=1.0,
                    )
                if stages >= 4:
                    nc.sync.dma_start(
                        updT_o[:].rearrange("(k p) q -> p k q", p=128), upd[:])

    with tile.TileContext(nc) as tc:
        for rep in range(reps):
            emit_body(tc, rep)

    nc.compile()
    return nc


def _host_prep(inputs):
    import ml_dtypes

    X = np.ascontiguousarray(np.asarray(inputs["token_features"],
                                        dtype=np.float32))
    B = np.asarray(inputs["message_boundaries"]).astype(np.int64)
    starts, ends = B[:, 0], B[:, 1]

    XT = np.ascontiguousarray(X.T)                     # [H, T] f32
    XT8 = np.clip(XT, -240.0, 240.0).astype(ml_dtypes.float8_e4m3fn)

    w = {k: np.ascontiguousarray(np.asarray(inputs[k], dtype=np.float32))
         for k in ("Wq", "Wk", "Wv", "Wo", "Wi1")}
    b = {k: np.asarray(inputs[k], dtype=np.float32)
         for k in ("bq", "bk", "bv", "bo", "bi1", "bi2")}
    Wi2 = np.asarray(inputs["Wi2"], dtype=np.float32)  # [512, 1]

    # tokens t with t2m[t]==m form [ends[m-1], ends[m])
    mlo = np.concatenate([[0], ends[:-1]]).astype(np.float32)
    mhi = ends.astype(np.float32)

    common = {
        "x": X,
        "xt8": XT8,
        "wqk": np.ascontiguousarray(
            (w["Wq"].astype(np.float64) @ w["Wk"].T.astype(np.float64)
             / 32.0).astype(np.float32)),
        "wv": w["Wv"],
        "wo": w["Wo"],
        "wi1b": w["Wi1"].astype(ml_dtypes.bfloat16),
        "wi1x": w["Wi1"],
        "wi2s": np.ascontiguousarray(Wi2[:, 0].reshape(4, 128).T),
        "wi2r": np.ascontiguousarray(Wi2[:, 0].reshape(4, 128).T),
        "bi1s": np.ascontiguousarray(b["bi1"].reshape(4, 128).T),
        "bvs": np.ascontiguousarray(b["bv"].reshape(8, 128).T),
        "bos": np.ascontiguousarray(b["bo"].reshape(8, 128).T),
        "cbase": np.ascontiguousarray(
            np.tile(((np.arange(512) // 8) * 512).astype(np.float32)[None, :],
                    (128, 1))),
    }

    in_maps = []
    for c in range(NCORES):
        msgs = np.arange(c * MSG_PER_CORE, (c + 1) * MSG_PER_CORE)
        row_m = np.repeat(msgs, QPAD // MSG_PER_CORE)   # [512], 64 per msg
        qlo_row = mlo[row_m].astype(np.float32)
        qhi_row = mhi[row_m].astype(np.float32)
        m = dict(common)
        m["xtcb"] = np.ascontiguousarray(
            XT[:, c * TC:(c + 1) * TC]).astype(ml_dtypes.bfloat16)
        m["bstg"] = starts[msgs].astype(np.float32).reshape(8, 1)
        m["qlo"] = np.ascontiguousarray(qlo_row.reshape(4, 128).T)
        m["qhi"] = np.ascontiguousarray(qhi_row.reshape(4, 128).T)
        in_maps.append(m)
    return in_maps


def _numpy_fallback(inputs):
    """Reference semantics in numpy (only for non-equal-length boundaries,
    which the stated harness never produces)."""
    X = np.asarray(inputs["token_features"], dtype=np.float32)
    B = np.asarray(inputs["message_boundaries"]).astype(np.int64)
    imp = (np.maximum(X @ np.asarray(inputs["Wi1"]) + np.asarray(inputs["bi1"]),
                      0) @ np.asarray(inputs["Wi2"])
           + np.asarray(inputs["bi2"]))[:, 0]
    k_imp = max(1, int((T // M) * 0.1))
    impm = imp.reshape(M, T // M)
    top_local = np.argsort(-impm, axis=1, kind="stable")[:, :k_imp]
    qidx_raw = (top_local + B[:, 0:1]).reshape(-1)
    qidx = np.minimum(qidx_raw, T - 1)
    Qp = X[qidx] @ np.asarray(inputs["Wq"]) + np.asarray(inputs["bq"])
    Km = X @ np.asarray(inputs["Wk"]) + np.asarray(inputs["bk"])
    S = (Qp @ Km.T) / np.float32(32.0)
    t2m = np.searchsorted(B[:, 1], np.arange(T), side="right")
    msg_ids = np.repeat(np.arange(M), k_imp)
    S[msg_ids[:, None] == t2m[None, :]] = -np.inf
    k_conn = min(5, T // M)
    top_idx = np.argsort(-S, axis=1, kind="stable")[:, :k_conn]
    tv = np.take_along_axis(S, top_idx, axis=1)
    e = np.exp(tv - tv[:, :1])
    attn = e / e.sum(1, keepdims=True)
    V = X @ np.asarray(inputs["Wv"]) + np.asarray(inputs["bv"])
    att = np.einsum("qk,qkh->qh", attn.astype(np.float32), V[top_idx])
    upd = att @ np.asarray(inputs["Wo"]) + np.asarray(inputs["bo"])
    out = X.copy()
    ok = qidx_raw < T
    np.add.at(out, qidx_raw[ok], upd[ok].astype(np.float32))
    return out


def kernel(**inputs):
    X = np.asarray(inputs["token_features"])
    B = np.asarray(inputs["message_boundaries"]).astype(np.int64)
    assert X.shape == (T, H), X.shape

    eq = (np.array_equal(B[:, 0], np.arange(M) * L)
          and np.array_equal(B[:, 1], (np.arange(M) + 1) * L))
    if not eq or np.any(np.asarray(inputs["bq"])):
        # bq != 0 would invalidate the host-fused W_qk = Wq WkT / 32
        return _numpy_fallback(inputs)

    from concourse.bass_utils import run_bass_kernel_spmd

    if "nc" not in _cache:
        _cache["nc"] = _build_program(
            int(os.environ.get("KERNEL_STAGES", "99")),
            int(os.environ.get("KERNEL_SUB", "9")),
            int(os.environ.get("KERNEL_REPS", "1")))
    nc = _cache["nc"]

    in_maps = _host_prep(inputs)
    trace = bool(int(os.environ.get("KERNEL_PROFILE", "0")))
    res = run_bass_kernel_spmd(nc, in_maps, list(range(NCORES)), trace=trace)
    if trace:
        _cache["exec_time_ns"] = res.exec_time_ns
        _cache["mean_exec_time_ns"] = res.mean_exec_time_ns

    out = np.array(X, dtype=np.float32, copy=True)
    idx_parts, upd_parts = [], []
    for c in range(NCORES):
        qi = res.results[c]["qidx_o"].astype(np.int64)
        valid = qi < T
        idx_parts.append(qi[valid])
        upd_parts.append(res.results[c]["updT_o"].T[valid].astype(np.float32))
    all_idx = np.concatenate(idx_parts)
    all_upd = np.concatenate(upd_parts)
    if len(np.unique(all_idx)) == len(all_idx):
        out[all_idx] += all_upd
    else:
        np.add.at(out, all_idx, all_upd)
    return out


# revision 33
# speedup vs baseline: 1.3981x; 1.3107x over previous
"""CrossMessageTokenAttention Trainium2 kernel (8 NeuronCores, SPMD).

Contract: kernel(**inputs) takes the FULL inputs of reference.setup_inputs()
and returns the FULL [32768, 1024] float32 output.

Strategy (hardcoded for T=32768, H=1024, M=64 messages, L=512, k_imp=51,
k_conn=5, 8 cores):
  - Queries (64 msgs x 51 selected tokens) are sharded by message: core c
    owns messages [8c, 8c+8) = 408 queries (padded to 512). No collectives.
  - Algebra: scores = ((Xq@Wq+bq)/32) @ Wk^T @ X^T  (bk dropped: a per-row
    shift, invariant for top-k and softmax). Values never materialized:
    attended = (sum_k attn_k X[idx_k]) @ Wv + bv, then @ Wo + bo.
  - Importance screen runs in bf16 (error ~1e-2 sigma vs a ~1.5e-1 sigma
    top-64 margin); the exact top-51 threshold is rescued in true fp32 on
    the 512 gathered candidate rows (stage 2b). 2b's PE part is emitted
    after phase B and its DVE sort after phase C, so both hide under
    phases that don't use those engines.
  - B = Xq @ (Wq Wk^T/32) in ONE exact-fp32 matmul stage; W_qk is
    precomputed on the host in f64 (valid since bq == 0, guarded).
    f32r here would add ~8e-3 abs score noise and flip top-5 membership
    at near-ties (~100 rows); fp32 is required.
  - The dominant [512,1024]x[1024,32768] score matmul runs in fp8e4
    DoubleRow (2 contraction rows/cycle, 2x bf16) as a SCREEN: per
    512-token block, HW top-8 (vector.max/max_index) read from PSUM.
  - Same-message masking on candidates by global-index range compare (each
    message is one aligned 512-token block).
  - Merge of the 512 candidates: quantized-score composite key
    q*32768+gidx (exact in f32 ints <= 2^24), two max8 passes -> top-16;
    index recovered by residue (robust to round-vs-trunc casts). Top-16
    (not 8): fp8 screen noise ~0.05 + 1/32 quantization push a true
    top-5 past rank 8 for ~2% of queries (measured 61/3264); at 16 it is
    0/3264.
  - Exact rescue: gather the 16 candidate rows in f32 (bf16 rows would add
    ~0.1 abs score noise, comparable to top-5 gaps), recompute their
    scores in fp32 on the vector engine against B, rank + softmax on the
    rescored values, weighted-sum the gathered rows.
  - Device outputs per core: updates^T [1024, 512] and query indices [512]
    (408 valid). The host scatters updates into a copy of token_features.
"""
import os

os.environ.setdefault("MYCRO_LOCAL_CACHE", "1")

import numpy as np

T, H = 32768, 1024
M, L = 64, 512
K_IMP = 51
NCORES = 8
MSG_PER_CORE = M // NCORES          # 8
QC = MSG_PER_CORE * K_IMP           # 408 queries per core
QPAD = 512
NQCH = QPAD // 128                  # 4
KCH = H // 128                      # 8
NTB = T // 512                      # 64 score blocks
TC = T // NCORES                    # 4096 tokens per core block
NEG = -1.0e30

_cache = {}


def _build_program(stages=99, sub=9, reps=1):
    import concourse.bacc as bacc
    import concourse.bass as bass
    import concourse.mybir as mybir
    import concourse.tile as tile
    from concourse.masks import make_identity

    F32 = mybir.dt.float32
    F32R = mybir.dt.float32r
    BF16 = mybir.dt.bfloat16
    FP8 = mybir.dt.float8e4
    U32 = mybir.dt.uint32
    AOP = mybir.AluOpType
    ACT = mybir.ActivationFunctionType
    AXX = mybir.AxisListType.X
    DR = mybir.MatmulPerfMode.DoubleRow

    nc = bacc.Bacc(None, target_bir_lowering=False, debug=False,
                   num_devices=NCORES)

    # ---------------- DRAM I/O ----------------
    x_d = nc.declare_dram_parameter("x", [T, H], F32, isOutput=False)
    xt8_d = nc.declare_dram_parameter("xt8", [H, T], FP8, isOutput=False)
    xtcb_d = nc.declare_dram_parameter("xtcb", [H, TC], BF16, isOutput=False)
    wqk_d = nc.declare_dram_parameter("wqk", [H, H], F32, isOutput=False)
    wv_d = nc.declare_dram_parameter("wv", [H, H], F32R, isOutput=False)
    wo_d = nc.declare_dram_parameter("wo", [H, H], F32R, isOutput=False)
    wi1b_d = nc.declare_dram_parameter("wi1b", [H, H // 2], BF16, isOutput=False)
    wi1x_d = nc.declare_dram_parameter("wi1x", [H, H // 2], F32, isOutput=False)
    wi2r_d = nc.declare_dram_parameter("wi2r", [128, 4], F32R, isOutput=False)
    wi2s_d = nc.declare_dram_parameter("wi2s", [128, 4], F32, isOutput=False)
    bi1s_d = nc.declare_dram_parameter("bi1s", [128, 4], F32, isOutput=False)
    bvs_d = nc.declare_dram_parameter("bvs", [128, 8], F32, isOutput=False)
    bos_d = nc.declare_dram_parameter("bos", [128, 8], F32, isOutput=False)
    bstg_d = nc.declare_dram_parameter("bstg", [8, 1], F32, isOutput=False)
    qlo_d = nc.declare_dram_parameter("qlo", [128, 4], F32, isOutput=False)
    qhi_d = nc.declare_dram_parameter("qhi", [128, 4], F32, isOutput=False)
    cbase_d = nc.declare_dram_parameter("cbase", [128, 512], F32, isOutput=False)

    updT_o = nc.declare_dram_parameter("updT_o", [H, QPAD], F32, isOutput=True)
    qidx_o = nc.declare_dram_parameter("qidx_o", [QPAD], U32, isOutput=True)

    wqk_r = wqk_d[:].rearrange("(k p) j -> p k j", p=128)
    wv_r = wv_d[:].rearrange("(k p) j -> p k j", p=128)
    wo_r = wo_d[:].rearrange("(k p) j -> p k j", p=128)

    def emit_body(tc, rep):
        with (
            tc.tile_pool(name=f"glob{rep}", bufs=1) as glob,
            tc.tile_pool(name=f"wsl{rep}", bufs=2) as wslp,
            tc.tile_pool(name=f"ps{rep}", bufs=6, space="PSUM") as ps,
            tc.tile_pool(name=f"pst{rep}", bufs=2, space="PSUM") as pst,
            tc.tile_pool(name=f"dram{rep}", bufs=1, space="DRAM") as dpool,
        ):
            ident = glob.tile([128, 128], F32, tag="ident")
            make_identity(nc, ident[:])
            b_nat = glob.tile([128, NQCH, H], F32, tag="bnat")
            # btb8 shares its slot with aT (btb8 dead after screening)
            btb8 = glob.tile([128, KCH, QPAD], FP8, tag="btb_aT")
            cvals = [glob.tile([128, 512], F32, tag=f"cv{ci}", name=f"cv{ci}")
                     for ci in range(NQCH)]
            cidx = [glob.tile([128, 512], U32, tag=f"cxi{ci}", name=f"cxi{ci}")
                    for ci in range(NQCH)]
            wi1_s = glob.tile([128, KCH, 512], BF16, tag="wi1")
            wi2_s = glob.tile([128, 4], F32R, tag="wi2")
            wi2x_s = glob.tile([128, 4], F32, tag="wi2x")
            bi1_s = glob.tile([128, 4], F32, tag="bi1")
            bv_s = glob.tile([128, 8], F32, tag="bvs")
            bo_s = glob.tile([128, 8], F32, tag="bos")
            cbase = glob.tile([128, 512], F32, tag="cbase")
            qlo = glob.tile([128, 4], F32, tag="qlo")
            qhi = glob.tile([128, 4], F32, tag="qhi")
            bstg = glob.tile([8, 1], F32, tag="bstg")
            qfg = glob.tile([8, 64], F32, tag="qfg")
            xqT = glob.tile([128, KCH, QPAD], F32, tag="xqT")
            nc.sync.dma_start(wi1_s[:], wi1b_d[:].rearrange("(k p) j -> p k j", p=128))
            nc.sync.dma_start(wi2_s[:], wi2r_d[:])
            nc.sync.dma_start(wi2x_s[:], wi2s_d[:])
            nc.sync.dma_start(bi1_s[:], bi1s_d[:])
            nc.sync.dma_start(bv_s[:], bvs_d[:])
            nc.sync.dma_start(bo_s[:], bos_d[:])
            nc.sync.dma_start(cbase[:], cbase_d[:])
            nc.sync.dma_start(qlo[:], qlo_d[:])
            nc.sync.dma_start(qhi[:], qhi_d[:])
            nc.sync.dma_start(bstg[:], bstg_d[:])


            # ============ Phase A: imp MLP, top-51, Xq, QpT, B ============
            with tc.tile_pool(name=f"pA{rep}", bufs=2) as pA, \
                 tc.tile_pool(name=f"pA1{rep}", bufs=1) as pA1:
                # ---- Stage 1: importance screen (bf16) over own 4096 ----
                # per-message imp rows assembled by SBUF->SBUF copies
                imp8 = pA1.tile([8, 512], F32, tag="imp8")
                xtc_r = xtcb_d[:].rearrange("(k p) t -> p k t", p=128)
                for nt in range(8):
                    xtile = pA.tile([128, KCH, 512], BF16, tag="xstream")
                    nc.sync.dma_start(
                        xtile[:], xtc_r[:, :, nt * 512:(nt + 1) * 512])
                    hT = pA.tile([128, 4, 512], F32R, tag="hT")
                    for mp in range(4):
                        hps = ps.tile([128, 512], F32, tag="pp")
                        for k in range(KCH):
                            nc.tensor.matmul(
                                out=hps[:],
                                lhsT=wi1_s[:, k, mp * 128:(mp + 1) * 128],
                                rhs=xtile[:, k, :],
                                start=(k == 0), stop=(k == KCH - 1),
                            )
                        nc.scalar.activation(
                            out=hT[:, mp, :], in_=hps[:], func=ACT.Relu,
                            bias=bi1_s[:, mp:mp + 1], scale=1.0,
                        )
                    ips = ps.tile([1, 512], F32, tag="pp")
                    for kp in range(4):
                        nc.tensor.matmul(
                            out=ips[:], lhsT=wi2_s[:, kp:kp + 1],
                            rhs=hT[:, kp, :],
                            start=(kp == 0), stop=(kp == 3),
                        )
                    impb = pA.tile([1, 512], F32, tag="impb")
                    nc.vector.tensor_copy(impb[:], ips[:])
                    nc.sync.dma_start(imp8[nt:nt + 1, :], impb[:])

                # ---- Stage 2: approx top-64 candidates per message ----
                imp8w = pA1.tile([8, 512], F32, tag="imp8w")
                nc.vector.tensor_copy(imp8w[:], imp8[:])
                v64 = pA1.tile([8, 64], F32, tag="v64")
                i64 = pA1.tile([8, 64], U32, tag="i64")
                for r in range(8):
                    sl = slice(r * 8, r * 8 + 8)
                    nc.vector.max(out=v64[:, sl], in_=imp8w[:])
                    nc.vector.max_index(out=i64[:, sl], in_max=v64[:, sl],
                                        in_values=imp8[:])
                    if r < 7:
                        nc.vector.match_replace(
                            out=imp8w[:], in_to_replace=v64[:, sl],
                            in_values=imp8w[:], imm_value=NEG,
                        )
                # global query index (also kept in glob for the 2b export)
                nc.vector.tensor_copy(qfg[:], i64[:])       # u32 -> f32
                nc.vector.tensor_scalar(
                    out=qfg[:], in0=qfg[:], scalar1=bstg[:, :1],
                    scalar2=float(T - 1), op0=AOP.add, op1=AOP.min,
                )
                qu = pA1.tile([8, 64], U32, tag="qu")
                nc.vector.tensor_copy(qu[:], qfg[:])        # f32 -> u32

                # ---- Stage 3: gather Xq rows (exact f32),
                #      transpose to XqT (kept in F32 and F32R views) ----
                for ci in range(NQCH):
                    qt = pA.tile([128, 1], U32, tag="qix")
                    nc.sync.dma_start(qt[:], qu[2 * ci:2 * ci + 2, :])
                    xq = pA.tile([128, H], F32, tag="xq")
                    nc.gpsimd.indirect_dma_start(
                        out=xq[:], out_offset=None, in_=x_d[:],
                        in_offset=bass.IndirectOffsetOnAxis(
                            ap=qt[:, :1], axis=0),
                    )
                    for k in range(KCH):
                        tps = pst.tile([128, 128], F32, tag="tp")
                        nc.tensor.transpose(
                            out=tps[:], in_=xq[:, k * 128:(k + 1) * 128],
                            identity=ident[:],
                        )
                        nc.vector.tensor_copy(
                            xqT[:, k, ci * 128:(ci + 1) * 128], tps[:])

                # ---- Stage 4+5 fused: B[q,h] = Xq @ (Wq WkT / 32)
                #      (W_qk precomputed on host in f64; exact-fp32 matmul.
                #      Valid because bq == 0 -- guarded in kernel()) ----
                for hh in range(2):
                    wkh = pA1.tile([128, KCH, 512], F32, tag="wkh",
                                   name="wkh")
                    nc.sync.dma_start(
                        wkh[:], wqk_r[:, :, hh * 512:(hh + 1) * 512])
                    for ci in range(NQCH):
                        bps = ps.tile([128, 512], F32, tag="pp")
                        for k in range(KCH):
                            nc.tensor.matmul(
                                out=bps[:],
                                lhsT=xqT[:, k, ci * 128:(ci + 1) * 128],
                                rhs=wkh[:, k, :],
                                start=(k == 0), stop=(k == KCH - 1),
                            )
                        nc.scalar.copy(
                            out=b_nat[:, ci, hh * 512:(hh + 1) * 512],
                            in_=bps[:])

                # btb8[h,q] (fp8) = B^T via PE transpose
                for ci in range(NQCH):
                    for k in range(KCH):
                        tps = pst.tile([128, 128], F32, tag="tp")
                        nc.tensor.transpose(
                            out=tps[:],
                            in_=b_nat[:, ci, k * 128:(k + 1) * 128],
                            identity=ident[:],
                        )
                        nc.vector.tensor_copy(
                            btb8[:, k, ci * 128:(ci + 1) * 128], tps[:])

            # ============ Phase B: fp8 DoubleRow screen + block top-8 ======
            xt8_r = xt8_d[:].rearrange("(k p) t -> p k t", p=128)
            if stages < 2:
                for ci in range(NQCH):
                    nc.vector.memset(cvals[ci][:], 0.0)
                    nc.vector.memset(cidx[ci][:], 0)
            with tc.tile_pool(name=f"pB{rep}", bufs=2) as pB:
                for tb2 in range(NTB // 2 if stages >= 2 else 0):
                    xt = pB.tile([128, KCH, 1024], FP8, tag="xt8s")
                    nc.sync.dma_start(
                        xt[:], xt8_r[:, :, tb2 * 1024:(tb2 + 1) * 1024])
                    for sb in range(2):
                        tb = tb2 * 2 + sb
                        sl5 = slice(sb * 512, (sb + 1) * 512)
                        for ci in range(NQCH):
                            sps = ps.tile([128, 512], F32, tag="pp")
                            for j in range(KCH // 2):
                                nc.tensor.matmul(
                                    out=sps[:],
                                    lhsT=btb8[:, 2 * j:2 * j + 2,
                                              ci * 128:(ci + 1) * 128],
                                    rhs=xt[:, 2 * j:2 * j + 2, sl5],
                                    start=(j == 0), stop=(j == KCH // 2 - 1),
                                    perf_mode=DR,
                                )
                            sl = slice(tb * 8, tb * 8 + 8)
                            nc.vector.max(out=cvals[ci][:, sl], in_=sps[:])
                            nc.vector.max_index(out=cidx[ci][:, sl],
                                                in_max=cvals[ci][:, sl],
                                                in_values=sps[:])

            # === Stage 2b PE part (off critical path): exact cand. imp ===
            # Emitted after B so its PE work fills the vector-bound phase C;
            # the DVE sort + sentinel export is emitted after phase C.
            wi1x_s = glob.tile([128, KCH, 512], F32, tag="wi1x_xgB",
                               name="wi1xs")
            nc.sync.dma_start(
                wi1x_s[:], wi1x_d[:].rearrange("(k p) j -> p k j", p=128))
            hx = glob.tile([128, 4, QPAD], F32, tag="wi1", name="hx")
            impx_dram = dpool.tile([QPAD], F32, name="impx_dram")
            for mp in range(4):
                hps2 = ps.tile([128, QPAD], F32, tag="pp", name="hps2")
                for k in range(KCH):
                    nc.tensor.matmul(
                        out=hps2[:],
                        lhsT=wi1x_s[:, k, mp * 128:(mp + 1) * 128],
                        rhs=xqT[:, k, :],
                        start=(k == 0), stop=(k == KCH - 1),
                    )
                nc.scalar.activation(
                    out=hx[:, mp, :], in_=hps2[:], func=ACT.Relu,
                    bias=bi1_s[:, mp:mp + 1], scale=1.0,
                )
            ipx = ps.tile([1, QPAD], F32, tag="pp", name="ipx")
            for kp in range(4):
                nc.tensor.matmul(
                    out=ipx[:], lhsT=wi2x_s[:, kp:kp + 1],
                    rhs=hx[:, kp, :], start=(kp == 0), stop=(kp == 3),
                )
            impxb = glob.tile([1, QPAD], F32, tag="impxb")
            nc.scalar.copy(out=impxb[:], in_=ipx[:])
            nc.sync.dma_start(
                impx_dram[:].rearrange("(a b) -> a b", a=1), impxb[:])

            # ============ Phase C: merge, exact rescue, attended ============
            aT = glob.tile([128, KCH, QPAD], F32R, tag="btb_aT")
            if stages < 3:
                nc.vector.memset(aT[:], 0.0)
            with tc.tile_pool(name=f"pC{rep}", bufs=2) as pC, \
                 tc.tile_pool(name=f"pCm{rep}", bufs=1) as pCm, \
                 tc.tile_pool(name=f"xg{rep}", bufs=2) as xgp:
                for ci in range(NQCH if stages >= 3 else 0):
                    gidx = pCm.tile([128, 512], F32, tag="gidx")
                    nc.vector.tensor_copy(gidx[:], cidx[ci][:])   # u32->f32
                    nc.vector.tensor_tensor(out=gidx[:], in0=gidx[:],
                                            in1=cbase[:], op=AOP.add)
                    # mask own-message candidates
                    t0 = pCm.tile([128, 512], F32, tag="t0")
                    nc.vector.tensor_scalar(
                        out=t0[:], in0=gidx[:], scalar1=qlo[:, ci:ci + 1],
                        scalar2=None, op0=AOP.is_ge)
                    t1 = pCm.tile([128, 512], F32, tag="t1")
                    nc.vector.tensor_scalar(
                        out=t1[:], in0=gidx[:], scalar1=qhi[:, ci:ci + 1],
                        scalar2=None, op0=AOP.is_lt)
                    nc.vector.tensor_tensor(out=t0[:], in0=t0[:], in1=t1[:],
                                            op=AOP.mult)
                    cm = pCm.tile([128, 512], F32, tag="cm")
                    nc.vector.scalar_tensor_tensor(
                        out=cm[:], in0=t0[:], scalar=NEG, in1=cvals[ci][:],
                        op0=AOP.mult, op1=AOP.add,
                    )
                    # composite key: quantized score (9 bits) * 32768 + gidx
                    cq = pCm.tile([128, 512], F32, tag="cq")
                    nc.vector.tensor_scalar(
                        out=cq[:], in0=cm[:], scalar1=-8.0, scalar2=7.96875,
                        op0=AOP.max, op1=AOP.min)
                    nc.vector.tensor_scalar(
                        out=cq[:], in0=cq[:], scalar1=8.0, scalar2=32.0,
                        op0=AOP.add, op1=AOP.mult)
                    cqu = pCm.tile([128, 512], U32, tag="cqu")
                    nc.vector.tensor_copy(cqu[:], cq[:])
                    nc.vector.tensor_copy(cq[:], cqu[:])
                    comp = pCm.tile([128, 512], F32, tag="comp")
                    nc.vector.scalar_tensor_tensor(
                        out=comp[:], in0=cq[:], scalar=32768.0, in1=gidx[:],
                        op0=AOP.mult, op1=AOP.add,
                    )
                    # merged top-16 (fp8 screen noise ~0.05 + 1/32 quant can
                    # push a true-top-5 past rank 8 among the 512 candidates;
                    # measured 61/3264 at top-8, 0/3264 at top-16)
                    m16 = pC.tile([128, 16], F32, tag="m16")
                    nc.vector.max(out=m16[:, 0:8], in_=comp[:])
                    nc.vector.match_replace(
                        out=comp[:], in_to_replace=m16[:, 0:8],
                        in_values=comp[:], imm_value=NEG,
                    )
                    nc.vector.max(out=m16[:, 8:16], in_=comp[:])
                    # recover gidx = m16 mod 32768 (robust to round-vs-trunc)
                    dq = pC.tile([128, 16], F32, tag="dq")
                    nc.vector.tensor_scalar(
                        out=dq[:], in0=m16[:], scalar1=1.0 / 32768.0,
                        scalar2=None, op0=AOP.mult)
                    dqu = pC.tile([128, 16], U32, tag="dqu")
                    nc.vector.tensor_copy(dqu[:], dq[:])
                    nc.vector.tensor_copy(dq[:], dqu[:])
                    idx16f = pC.tile([128, 16], F32, tag="idx16f")
                    nc.vector.scalar_tensor_tensor(
                        out=idx16f[:], in0=dq[:], scalar=-32768.0, in1=m16[:],
                        op0=AOP.mult, op1=AOP.add,
                    )
                    neg = pC.tile([128, 16], F32, tag="negf")
                    nc.vector.tensor_scalar(
                        out=neg[:], in0=idx16f[:], scalar1=0.0, scalar2=32768.0,
                        op0=AOP.is_lt, op1=AOP.mult)
                    nc.vector.tensor_tensor(out=idx16f[:], in0=idx16f[:],
                                            in1=neg[:], op=AOP.add)
                    idx16u = pC.tile([128, 16], U32, tag="idx16u")
                    nc.vector.tensor_copy(idx16u[:], idx16f[:])

                    # gather candidate rows (f32, one indirect DMA per
                    # row -- multi-row offset APs abort on device) + exact
                    # fp32 rescore. Two 4-row groups live in the xg pool;
                    # two borrow the xqT and wi1x glob slots (dead once
                    # stage 2b's PE part ran).
                    xgg = [xgp.tile([128, 4, H], F32, tag=f"xgg{g}",
                                    name=f"xgg{g}") for g in range(2)]
                    xgg.append(glob.tile([128, 4, H], F32, tag="xqT",
                                         name=f"xgA{ci}"))
                    xgg.append(glob.tile([128, 4, H], F32, tag="wi1x_xgB",
                                         name=f"xgB{ci}"))
                    rows = [xgg[r // 4][:, r % 4, :] for r in range(16)]
                    s16 = pC.tile([128, 16], F32, tag="s16")
                    for r in range(16):
                        if sub >= 1:
                            nc.gpsimd.indirect_dma_start(
                                out=rows[r], out_offset=None, in_=x_d[:],
                                in_offset=bass.IndirectOffsetOnAxis(
                                    ap=idx16u[:, r:r + 1], axis=0),
                            )
                        else:
                            nc.vector.memset(rows[r], 0.5)
                        if sub >= 2:
                            scr = pC.tile([128, H], F32, tag="scr")
                            nc.vector.scalar_tensor_tensor(
                                out=scr[:], in0=b_nat[:, ci, :], scalar=1.0,
                                in1=rows[r], op0=AOP.mult, op1=AOP.mult,
                                accum_out=s16[:, r:r + 1],
                            )
                    if sub < 2:
                        nc.vector.tensor_copy(s16[:], m16[:])

                    # exact top-5 softmax over the 16 rescored candidates
                    srt8 = pC.tile([128, 8], F32, tag="srt8")
                    nc.vector.max(out=srt8[:], in_=s16[:])
                    nmax = pC.tile([128, 1], F32, tag="nmax")
                    nc.vector.tensor_scalar(
                        out=nmax[:], in0=srt8[:, 0:1], scalar1=-1.0,
                        scalar2=None, op0=AOP.mult)
                    e16 = pC.tile([128, 16], F32, tag="e16")
                    nc.scalar.activation(out=e16[:], in_=s16[:], func=ACT.Exp,
                                         bias=nmax[:, :1], scale=1.0)
                    msk = pC.tile([128, 16], F32, tag="msk")
                    nc.vector.tensor_scalar(
                        out=msk[:], in0=s16[:], scalar1=srt8[:, 4:5],
                        scalar2=None, op0=AOP.is_ge)
                    zsum = pC.tile([128, 1], F32, tag="zsum")
                    nc.vector.scalar_tensor_tensor(
                        out=e16[:], in0=e16[:], scalar=1.0, in1=msk[:],
                        op0=AOP.mult, op1=AOP.mult, accum_out=zsum[:, :1],
                    )
                    rz = pC.tile([128, 1], F32, tag="rz")
                    if sub >= 3:
                        nc.vector.reciprocal(out=rz[:], in_=zsum[:])
                    else:
                        nc.vector.memset(rz[:], 0.2)
                    attn = pC.tile([128, 16], F32, tag="attn")
                    nc.vector.tensor_scalar(
                        out=attn[:], in0=e16[:], scalar1=rz[:, :1],
                        scalar2=None, op0=AOP.mult)

                    # attended = sum_r attn_r * row_r ; transpose into aT
                    acc = pC.tile([128, H], F32, tag="acc")
                    nc.vector.tensor_scalar(
                        out=acc[:], in0=rows[0], scalar1=attn[:, 0:1],
                        scalar2=None, op0=AOP.mult)
                    for r in range(1, 16):
                        nc.vector.scalar_tensor_tensor(
                            out=acc[:], in0=rows[r], scalar=attn[:, r:r + 1],
                            in1=acc[:], op0=AOP.mult, op1=AOP.add,
                        )
                    for k in range(KCH):
                        tps = pst.tile([128, 128], F32, tag="tp")
                        nc.tensor.transpose(
                            out=tps[:], in_=acc[:, k * 128:(k + 1) * 128],
                            identity=ident[:],
                        )
                        nc.vector.tensor_copy(
                            aT[:, k, ci * 128:(ci + 1) * 128], tps[:])

            # ====== Stage 2b sort + sentinel export (DVE, tail) ======
            with tc.tile_pool(name=f"p2b{rep}", bufs=1) as p2b:
                vx = p2b.tile([8, 64], F32, tag="vx")
                nc.sync.dma_start(
                    vx[:], impx_dram[:].rearrange("(m t) -> m t", m=8))
                vxw = p2b.tile([8, 64], F32, tag="vxw")
                nc.vector.tensor_copy(vxw[:], vx[:])
                v8r = p2b.tile([8, 8], F32, tag="v8r")
                for r in range(6):
                    nc.vector.max(out=v8r[:], in_=vxw[:])
                    nc.vector.match_replace(
                        out=vxw[:], in_to_replace=v8r[:],
                        in_values=vxw[:], imm_value=NEG,
                    )
                nc.vector.max(out=v8r[:], in_=vxw[:])   # ranks 49..56
                # theta = exact 51st (col 2); valid = exact imp >= theta
                vm = p2b.tile([8, 64], F32, tag="vm")
                nc.vector.tensor_scalar(
                    out=vm[:], in0=vx[:], scalar1=v8r[:, 2:3], scalar2=None,
                    op0=AOP.is_ge)
                qsent = p2b.tile([8, 64], F32, tag="qsent")
                nc.vector.tensor_tensor(out=qsent[:], in0=qfg[:], in1=vm[:],
                                        op=AOP.mult)
                sent2 = p2b.tile([8, 64], F32, tag="sent2")
                nc.vector.tensor_scalar(
                    out=sent2[:], in0=vm[:], scalar1=-1.0e9, scalar2=1.0e9,
                    op0=AOP.mult, op1=AOP.add)
                nc.vector.tensor_tensor(out=qsent[:], in0=qsent[:],
                                        in1=sent2[:], op=AOP.add)
                qsu = p2b.tile([8, 64], U32, tag="qsu")
                nc.vector.tensor_copy(qsu[:], qsent[:])
                nc.sync.dma_start(
                    qidx_o[:].rearrange("(m t) -> m t", m=8), qsu[:])

            # ============ Phase D: output projections (f32r) ============
            if stages < 4:
                with tc.tile_pool(name=f"pDz{rep}", bufs=1) as pDz:
                    z = pDz.tile([128, KCH, QPAD], F32, tag="z")
                    nc.vector.memset(z[:], 0.0)
                    nc.sync.dma_start(
                        updT_o[:].rearrange("(k p) q -> p k q", p=128), z[:])
            with tc.tile_pool(name=f"pD{rep}", bufs=1) as pD:
                vT = pD.tile([128, KCH, QPAD], F32R, tag="vT")
                for mp in range(KCH if stages >= 4 else 0):
                    wsl = wslp.tile([128, KCH, 128], F32R, tag="wslr", name="wslr")
                    nc.sync.dma_start(
                        wsl[:], wv_r[:, :, mp * 128:(mp + 1) * 128])
                    vps = ps.tile([128, QPAD], F32, tag="pp")
                    for k in range(KCH):
                        nc.tensor.matmul(
                            out=vps[:], lhsT=wsl[:, k, :], rhs=aT[:, k, :],
                            start=(k == 0), stop=(k == KCH - 1),
                        )
                    nc.scalar.activation(
                        out=vT[:, mp, :], in_=vps[:], func=ACT.Identity,
                        bias=bv_s[:, mp:mp + 1], scale=1.0,
                    )
                upd = pD.tile([128, KCH, QPAD], F32, tag="upd")
                for mp in range(KCH if stages >= 4 else 0):
                    wsl = wslp.tile([128, KCH, 128], F32R, tag="wslr", name="wslr")
                    nc.sync.dma_start(
                        wsl[:], wo_r[:, :, mp * 128:(mp + 1) * 128])
                    ups = ps.tile([128, QPAD], F32, tag="pp")
                    for k in range(KCH):
                        nc.tensor.matmul(
                            out=ups[:], lhsT=wsl[:, k, :], rhs=vT[:, k, :],
                            start=(k == 0), stop=(k == KCH - 1),
                        )
                    nc.scalar.activation(
                        out=upd[:, mp, :], in_=ups[:], func=ACT.Identity,
                        bias=bo_s[:, mp:mp + 1], scale=1.0,
                    )
                if stages >= 4:
                    nc.sync.dma_start(
                        updT_o[:].rearrange("(k p) q -> p k q", p=128), upd[:])

    with tile.TileContext(nc) as tc:
        for rep in range(reps):
            emit_body(tc, rep)

    nc.compile()
    return nc


def _host_prep(inputs):
    import ml_dtypes

    X = np.ascontiguousarray(np.asarray(inputs["token_features"],
                                        dtype=np.float32))
    B = np.asarray(inputs["message_boundaries"]).astype(np.int64)
    starts, ends = B[:, 0], B[:, 1]

    XT = np.ascontiguousarray(X.T)                     # [H, T] f32
    XT8 = np.clip(XT, -240.0, 240.0).astype(ml_dtypes.float8_e4m3fn)

    w = {k: np.ascontiguousarray(np.asarray(inputs[k], dtype=np.float32))
         for k in ("Wq", "Wk", "Wv", "Wo", "Wi1")}
    b = {k: np.asarray(inputs[k], dtype=np.float32)
         for k in ("bq", "bk", "bv", "bo", "bi1", "bi2")}
    Wi2 = np.asarray(inputs["Wi2"], dtype=np.float32)  # [512, 1]

    # tokens t with t2m[t]==m form [ends[m-1], ends[m])
    mlo = np.concatenate([[0], ends[:-1]]).astype(np.float32)
    mhi = ends.astype(np.float32)

    common = {
        "x": X,
        "xt8": XT8,
        "wqk": np.ascontiguousarray(
            (w["Wq"].astype(np.float64) @ w["Wk"].T.astype(np.float64)
             / 32.0).astype(np.float32)),
        "wv": w["Wv"],
        "wo": w["Wo"],
        "wi1b": w["Wi1"].astype(ml_dtypes.bfloat16),
        "wi1x": w["Wi1"],
        "wi2s": np.ascontiguousarray(Wi2[:, 0].reshape(4, 128).T),
        "wi2r": np.ascontiguousarray(Wi2[:, 0].reshape(4, 128).T),
        "bi1s": np.ascontiguousarray(b["bi1"].reshape(4, 128).T),
        "bvs": np.ascontiguousarray(b["bv"].reshape(8, 128).T),
        "bos": np.ascontiguousarray(b["bo"].reshape(8, 128).T),
        "cbase": np.ascontiguousarray(
            np.tile(((np.arange(512) // 8) * 512).astype(np.float32)[None, :],
                    (128, 1))),
    }

    in_maps = []
    for c in range(NCORES):
        msgs = np.arange(c * MSG_PER_CORE, (c + 1) * MSG_PER_CORE)
        row_m = np.repeat(msgs, QPAD // MSG_PER_CORE)   # [512], 64 per msg
        qlo_row = mlo[row_m].astype(np.float32)
        qhi_row = mhi[row_m].astype(np.float32)
        m = dict(common)
        m["xtcb"] = np.ascontiguousarray(
            XT[:, c * TC:(c + 1) * TC]).astype(ml_dtypes.bfloat16)
        m["bstg"] = starts[msgs].astype(np.float32).reshape(8, 1)
        m["qlo"] = np.ascontiguousarray(qlo_row.reshape(4, 128).T)
        m["qhi"] = np.ascontiguousarray(qhi_row.reshape(4, 128).T)
        in_maps.append(m)
    return in_maps


def _numpy_fallback(inputs):
    """Reference semantics in numpy (only for non-equal-length boundaries,
    which the stated harness never produces)."""
    X = np.asarray(inputs["token_features"], dtype=np.float32)
    B = np.asarray(inputs["message_boundaries"]).astype(np.int64)
    imp = (np.maximum(X @ np.asarray(inputs["Wi1"]) + np.asarray(inputs["bi1"]),
                      0) @ np.asarray(inputs["Wi2"])
           + np.asarray(inputs["bi2"]))[:, 0]
    k_imp = max(1, int((T // M) * 0.1))
    impm = imp.reshape(M, T // M)
    top_local = np.argsort(-impm, axis=1, kind="stable")[:, :k_imp]
    qidx_raw = (top_local + B[:, 0:1]).reshape(-1)
    qidx = np.minimum(qidx_raw, T - 1)
    Qp = X[qidx] @ np.asarray(inputs["Wq"]) + np.asarray(inputs["bq"])
    Km = X @ np.asarray(inputs["Wk"]) + np.asarray(inputs["bk"])
    S = (Qp @ Km.T) / np.float32(32.0)
    t2m = np.searchsorted(B[:, 1], np.arange(T), side="right")
    msg_ids = np.repeat(np.arange(M), k_imp)
    S[msg_ids[:, None] == t2m[None, :]] = -np.inf
    k_conn = min(5, T // M)
    top_idx = np.argsort(-S, axis=1, kind="stable")[:, :k_conn]
    tv = np.take_along_axis(S, top_idx, axis=1)
    e = np.exp(tv - tv[:, :1])
    attn = e / e.sum(1, keepdims=True)
    V = X @ np.asarray(inputs["Wv"]) + np.asarray(inputs["bv"])
    att = np.einsum("qk,qkh->qh", attn.astype(np.float32), V[top_idx])
    upd = att @ np.asarray(inputs["Wo"]) + np.asarray(inputs["bo"])
    out = X.copy()
    ok = qidx_raw < T
    np.add.at(out, qidx_raw[ok], upd[ok].astype(np.float32))
    return out


def kernel(**inputs):
    X = np.asarray(inputs["token_features"])
    B = np.asarray(inputs["message_boundaries"]).astype(np.int64)
    assert X.shape == (T, H), X.shape

    eq = (np.array_equal(B[:, 0], np.arange(M) * L)
          and np.array_equal(B[:, 1], (np.arange(M) + 1) * L))
    if not eq or np.any(np.asarray(inputs["bq"])):
        # bq != 0 would invalidate the host-fused W_qk = Wq WkT / 32
        return _numpy_fallback(inputs)

    from concourse.bass_utils import run_bass_kernel_spmd

    if "nc" not in _cache:
        _cache["nc"] = _build_program(
            int(os.environ.get("KERNEL_STAGES", "99")),
            int(os.environ.get("KERNEL_SUB", "9")),
            int(os.environ.get("KERNEL_REPS", "1")))
    nc = _cache["nc"]

    in_maps = _host_prep(inputs)
    trace = bool(int(os.environ.get("KERNEL_PROFILE", "0")))
    res = run_bass_kernel_spmd(nc, in_maps, list(range(NCORES)), trace=trace)
    if trace:
        _cache["exec_time_ns"] = res.exec_time_ns
        _cache["mean_exec_time_ns"] = res.mean_exec_time_ns

    out = np.array(X, dtype=np.float32, copy=True)
    idx_parts, upd_parts = [], []
    for c in range(NCORES):
        qi = res.results[c]["qidx_o"].astype(np.int64)
        valid = qi < T
        idx_parts.append(qi[valid])
        upd_parts.append(res.results[c]["updT_o"].T[valid].astype(np.float32))
    all_idx = np.concatenate(idx_parts)
    all_upd = np.concatenate(upd_parts)
    if len(np.unique(all_idx)) == len(all_idx):
        out[all_idx] += all_upd
    else:
        np.add.at(out, all_idx, all_upd)
    return out


# revision 35
# speedup vs baseline: 1.4186x; 1.0147x over previous
"""CrossMessageTokenAttention Trainium2 kernel (8 NeuronCores, SPMD).

Contract: kernel(**inputs) takes the FULL inputs of reference.setup_inputs()
and returns the FULL [32768, 1024] float32 output.

Strategy (hardcoded for T=32768, H=1024, M=64 messages, L=512, k_imp=51,
k_conn=5, 8 cores):
  - Queries (64 msgs x 51 selected tokens) are sharded by message: core c
    owns messages [8c, 8c+8) = 408 queries (padded to 512). No collectives.
  - Algebra: scores = ((Xq@Wq+bq)/32) @ Wk^T @ X^T  (bk dropped: a per-row
    shift, invariant for top-k and softmax). Values never materialized:
    attended = (sum_k attn_k X[idx_k]) @ Wv + bv, then @ Wo + bo.
  - Importance screen runs in bf16 (error ~1e-2 sigma vs a ~1.5e-1 sigma
    top-64 margin); the exact top-51 threshold is rescued in true fp32 on
    the 512 gathered candidate rows (stage 2b). 2b's PE part is emitted
    after phase B and its DVE sort after phase C, so both hide under
    phases that don't use those engines.
  - B = Xq @ (Wq Wk^T/32) in ONE exact-fp32 matmul stage; W_qk is
    precomputed on the host in f64 (valid since bq == 0, guarded).
    f32r here would add ~8e-3 abs score noise and flip top-5 membership
    at near-ties (~100 rows); fp32 is required.
  - The dominant [512,1024]x[1024,32768] score matmul runs in fp8e4
    DoubleRow (2 contraction rows/cycle, 2x bf16) as a SCREEN: per
    512-token block, HW top-8 (vector.max/max_index) read from PSUM.
  - Same-message masking on candidates by global-index range compare (each
    message is one aligned 512-token block).
  - Merge of the 512 candidates: quantized-score composite key
    q*32768+gidx (exact in f32 ints <= 2^24), two max8 passes -> top-16;
    index recovered by residue (robust to round-vs-trunc casts). Top-16
    (not 8): fp8 screen noise ~0.05 + 1/32 quantization push a true
    top-5 past rank 8 for ~2% of queries (measured 61/3264); at 16 it is
    0/3264.
  - Exact rescue: gather the 16 candidate rows in f32 (bf16 rows would add
    ~0.1 abs score noise, comparable to top-5 gaps), recompute their
    scores in fp32 on the vector engine against B, rank + softmax on the
    rescored values, weighted-sum the gathered rows.
  - Device outputs per core: updates^T [1024, 512] and query indices [512]
    (408 valid). The host scatters updates into a copy of token_features.
"""
import os

os.environ.setdefault("MYCRO_LOCAL_CACHE", "1")

import numpy as np

T, H = 32768, 1024
M, L = 64, 512
K_IMP = 51
NCORES = 8
MSG_PER_CORE = M // NCORES          # 8
QC = MSG_PER_CORE * K_IMP           # 408 queries per core
QPAD = 512
NQCH = QPAD // 128                  # 4
KCH = H // 128                      # 8
NTB = T // 512                      # 64 score blocks
TC = T // NCORES                    # 4096 tokens per core block
NEG = -1.0e30

_cache = {}


def _build_program(stages=99, sub=9, reps=1):
    import concourse.bacc as bacc
    import concourse.bass as bass
    import concourse.mybir as mybir
    import concourse.tile as tile
    from concourse.masks import make_identity

    F32 = mybir.dt.float32
    F32R = mybir.dt.float32r
    BF16 = mybir.dt.bfloat16
    FP8 = mybir.dt.float8e4
    U32 = mybir.dt.uint32
    AOP = mybir.AluOpType
    ACT = mybir.ActivationFunctionType
    AXX = mybir.AxisListType.X
    DR = mybir.MatmulPerfMode.DoubleRow

    nc = bacc.Bacc(None, target_bir_lowering=False, debug=False,
                   num_devices=NCORES)

    # ---------------- DRAM I/O ----------------
    x_d = nc.declare_dram_parameter("x", [T, H], F32, isOutput=False)
    xt8_d = nc.declare_dram_parameter("xt8", [H, T], FP8, isOutput=False)
    xtcb_d = nc.declare_dram_parameter("xtcb", [H, TC], BF16, isOutput=False)
    wqk_d = nc.declare_dram_parameter("wqk", [H, H], F32, isOutput=False)
    wv_d = nc.declare_dram_parameter("wv", [H, H], F32R, isOutput=False)
    wo_d = nc.declare_dram_parameter("wo", [H, H], F32R, isOutput=False)
    wi1b_d = nc.declare_dram_parameter("wi1b", [H, H // 2], BF16, isOutput=False)
    wi1x_d = nc.declare_dram_parameter("wi1x", [H, H // 2], F32, isOutput=False)
    wi2r_d = nc.declare_dram_parameter("wi2r", [128, 4], F32R, isOutput=False)
    wi2s_d = nc.declare_dram_parameter("wi2s", [128, 4], F32, isOutput=False)
    bi1s_d = nc.declare_dram_parameter("bi1s", [128, 4], F32, isOutput=False)
    bvs_d = nc.declare_dram_parameter("bvs", [128, 8], F32, isOutput=False)
    bos_d = nc.declare_dram_parameter("bos", [128, 8], F32, isOutput=False)
    bstg_d = nc.declare_dram_parameter("bstg", [8, 1], F32, isOutput=False)
    qlo_d = nc.declare_dram_parameter("qlo", [128, 4], F32, isOutput=False)
    qhi_d = nc.declare_dram_parameter("qhi", [128, 4], F32, isOutput=False)
    cbase_d = nc.declare_dram_parameter("cbase", [128, 512], F32, isOutput=False)

    updT_o = nc.declare_dram_parameter("updT_o", [H, QPAD], F32, isOutput=True)
    qidx_o = nc.declare_dram_parameter("qidx_o", [QPAD], U32, isOutput=True)

    wqk_r = wqk_d[:].rearrange("(k p) j -> p k j", p=128)
    wv_r = wv_d[:].rearrange("(k p) j -> p k j", p=128)
    wo_r = wo_d[:].rearrange("(k p) j -> p k j", p=128)

    def emit_body(tc, rep):
        with (
            tc.tile_pool(name=f"glob{rep}", bufs=1) as glob,
            tc.tile_pool(name=f"wsl{rep}", bufs=2) as wslp,
            tc.tile_pool(name=f"ps{rep}", bufs=6, space="PSUM") as ps,
            tc.tile_pool(name=f"pst{rep}", bufs=2, space="PSUM") as pst,
            tc.tile_pool(name=f"dram{rep}", bufs=1, space="DRAM") as dpool,
        ):
            ident = glob.tile([128, 128], F32, tag="ident")
            make_identity(nc, ident[:])
            b_nat = glob.tile([128, NQCH, H], F32, tag="bnat")
            # btb8 shares its slot with aT (btb8 dead after screening)
            btb8 = glob.tile([128, KCH, QPAD], FP8, tag="btb_aT")
            cvals = [glob.tile([128, 512], F32, tag=f"cv{ci}", name=f"cv{ci}")
                     for ci in range(NQCH)]
            cidx = [glob.tile([128, 512], U32, tag=f"cxi{ci}", name=f"cxi{ci}")
                    for ci in range(NQCH)]
            wi1_s = glob.tile([128, KCH, 512], BF16, tag="wi1")
            wi2_s = glob.tile([128, 4], F32R, tag="wi2")
            wi2x_s = glob.tile([128, 4], F32, tag="wi2x")
            bi1_s = glob.tile([128, 4], F32, tag="bi1")
            bv_s = glob.tile([128, 8], F32, tag="bvs")
            bo_s = glob.tile([128, 8], F32, tag="bos")
            cbase = glob.tile([128, 512], F32, tag="cbase")
            qlo = glob.tile([128, 4], F32, tag="qlo")
            qhi = glob.tile([128, 4], F32, tag="qhi")
            bstg = glob.tile([8, 1], F32, tag="bstg")
            qfg = glob.tile([8, 64], F32, tag="qfg")
            xqT = glob.tile([128, KCH, QPAD], F32, tag="xqT")
            nc.sync.dma_start(wi1_s[:], wi1b_d[:].rearrange("(k p) j -> p k j", p=128))
            nc.sync.dma_start(wi2_s[:], wi2r_d[:])
            nc.sync.dma_start(wi2x_s[:], wi2s_d[:])
            nc.sync.dma_start(bi1_s[:], bi1s_d[:])
            nc.sync.dma_start(bv_s[:], bvs_d[:])
            nc.sync.dma_start(bo_s[:], bos_d[:])
            nc.sync.dma_start(cbase[:], cbase_d[:])
            nc.sync.dma_start(qlo[:], qlo_d[:])
            nc.sync.dma_start(qhi[:], qhi_d[:])
            nc.sync.dma_start(bstg[:], bstg_d[:])


            # ============ Phase A: imp MLP, top-51, Xq, QpT, B ============
            with tc.tile_pool(name=f"pA{rep}", bufs=2) as pA, \
                 tc.tile_pool(name=f"pA1{rep}", bufs=1) as pA1:
                # ---- Stage 1: importance screen (bf16) over own 4096 ----
                # per-message imp rows assembled by SBUF->SBUF copies
                imp8 = pA1.tile([8, 512], F32, tag="imp8")
                xtc_r = xtcb_d[:].rearrange("(k p) t -> p k t", p=128)
                for nt in range(8):
                    xtile = pA.tile([128, KCH, 512], BF16, tag="xstream")
                    nc.sync.dma_start(
                        xtile[:], xtc_r[:, :, nt * 512:(nt + 1) * 512])
                    hT = pA.tile([128, 4, 512], F32R, tag="hT")
                    for mp in range(4):
                        hps = ps.tile([128, 512], F32, tag="pp")
                        for k in range(KCH):
                            nc.tensor.matmul(
                                out=hps[:],
                                lhsT=wi1_s[:, k, mp * 128:(mp + 1) * 128],
                                rhs=xtile[:, k, :],
                                start=(k == 0), stop=(k == KCH - 1),
                            )
                        nc.scalar.activation(
                            out=hT[:, mp, :], in_=hps[:], func=ACT.Relu,
                            bias=bi1_s[:, mp:mp + 1], scale=1.0,
                        )
                    ips = ps.tile([1, 512], F32, tag="pp")
                    for kp in range(4):
                        nc.tensor.matmul(
                            out=ips[:], lhsT=wi2_s[:, kp:kp + 1],
                            rhs=hT[:, kp, :],
                            start=(kp == 0), stop=(kp == 3),
                        )
                    impb = pA.tile([1, 512], F32, tag="impb")
                    nc.vector.tensor_copy(impb[:], ips[:])
                    nc.sync.dma_start(imp8[nt:nt + 1, :], impb[:])

                # ---- Stage 2: approx top-64 candidates per message ----
                imp8w = pA1.tile([8, 512], F32, tag="imp8w")
                nc.vector.tensor_copy(imp8w[:], imp8[:])
                v64 = pA1.tile([8, 64], F32, tag="v64")
                i64 = pA1.tile([8, 64], U32, tag="i64")
                for r in range(8):
                    sl = slice(r * 8, r * 8 + 8)
                    nc.vector.max(out=v64[:, sl], in_=imp8w[:])
                    nc.vector.max_index(out=i64[:, sl], in_max=v64[:, sl],
                                        in_values=imp8[:])
                    if r < 7:
                        nc.vector.match_replace(
                            out=imp8w[:], in_to_replace=v64[:, sl],
                            in_values=imp8w[:], imm_value=NEG,
                        )
                # global query index (also kept in glob for the 2b export)
                nc.vector.tensor_copy(qfg[:], i64[:])       # u32 -> f32
                nc.vector.tensor_scalar(
                    out=qfg[:], in0=qfg[:], scalar1=bstg[:, :1],
                    scalar2=float(T - 1), op0=AOP.add, op1=AOP.min,
                )
                qu = pA1.tile([8, 64], U32, tag="qu")
                nc.vector.tensor_copy(qu[:], qfg[:])        # f32 -> u32

                # ---- Stage 3: gather Xq rows (exact f32),
                #      transpose to XqT (kept in F32 and F32R views) ----
                for ci in range(NQCH):
                    qt = pA.tile([128, 1], U32, tag="qix")
                    nc.sync.dma_start(qt[:], qu[2 * ci:2 * ci + 2, :])
                    xq = pA.tile([128, H], F32, tag="xq")
                    nc.gpsimd.indirect_dma_start(
                        out=xq[:], out_offset=None, in_=x_d[:],
                        in_offset=bass.IndirectOffsetOnAxis(
                            ap=qt[:, :1], axis=0),
                    )
                    for k in range(KCH):
                        tps = pst.tile([128, 128], F32, tag="tp")
                        nc.tensor.transpose(
                            out=tps[:], in_=xq[:, k * 128:(k + 1) * 128],
                            identity=ident[:],
                        )
                        nc.vector.tensor_copy(
                            xqT[:, k, ci * 128:(ci + 1) * 128], tps[:])

                # ---- Stage 4+5 fused: B[q,h] = Xq @ (Wq WkT / 32)
                #      (W_qk precomputed on host in f64; exact-fp32 matmul.
                #      Valid because bq == 0 -- guarded in kernel()) ----
                for hh in range(2):
                    wkh = pA1.tile([128, KCH, 512], F32, tag="wkh",
                                   name="wkh")
                    nc.sync.dma_start(
                        wkh[:], wqk_r[:, :, hh * 512:(hh + 1) * 512])
                    for ci in range(NQCH):
                        bps = ps.tile([128, 512], F32, tag="pp")
                        for k in range(KCH):
                            nc.tensor.matmul(
                                out=bps[:],
                                lhsT=xqT[:, k, ci * 128:(ci + 1) * 128],
                                rhs=wkh[:, k, :],
                                start=(k == 0), stop=(k == KCH - 1),
                            )
                        nc.scalar.copy(
                            out=b_nat[:, ci, hh * 512:(hh + 1) * 512],
                            in_=bps[:])

                # btb8[h,q] (fp8) = B^T via PE transpose
                for ci in range(NQCH):
                    for k in range(KCH):
                        tps = pst.tile([128, 128], F32, tag="tp")
                        nc.tensor.transpose(
                            out=tps[:],
                            in_=b_nat[:, ci, k * 128:(k + 1) * 128],
                            identity=ident[:],
                        )
                        nc.vector.tensor_copy(
                            btb8[:, k, ci * 128:(ci + 1) * 128], tps[:])

            # ============ Phase B: fp8 DoubleRow screen + block top-8 ======
            xt8_r = xt8_d[:].rearrange("(k p) t -> p k t", p=128)
            if stages < 2:
                for ci in range(NQCH):
                    nc.vector.memset(cvals[ci][:], 0.0)
                    nc.vector.memset(cidx[ci][:], 0)
            with tc.tile_pool(name=f"pB{rep}", bufs=2) as pB:
                for tb2 in range(NTB // 2 if stages >= 2 else 0):
                    xt = pB.tile([128, KCH, 1024], FP8, tag="xt8s")
                    nc.sync.dma_start(
                        xt[:], xt8_r[:, :, tb2 * 1024:(tb2 + 1) * 1024])
                    for sb in range(2):
                        tb = tb2 * 2 + sb
                        sl5 = slice(sb * 512, (sb + 1) * 512)
                        for ci in range(NQCH):
                            sps = ps.tile([128, 512], F32, tag="pp")
                            for j in range(KCH // 2):
                                nc.tensor.matmul(
                                    out=sps[:],
                                    lhsT=btb8[:, 2 * j:2 * j + 2,
                                              ci * 128:(ci + 1) * 128],
                                    rhs=xt[:, 2 * j:2 * j + 2, sl5],
                                    start=(j == 0), stop=(j == KCH // 2 - 1),
                                    perf_mode=DR,
                                )
                            sl = slice(tb * 8, tb * 8 + 8)
                            nc.vector.max(out=cvals[ci][:, sl], in_=sps[:])
                            nc.vector.max_index(out=cidx[ci][:, sl],
                                                in_max=cvals[ci][:, sl],
                                                in_values=sps[:])

            # === Stage 2b PE part (off critical path): exact cand. imp ===
            # Emitted after B so its PE work fills the vector-bound phase C;
            # the DVE sort + sentinel export is emitted after phase C.
            wi1x_s = glob.tile([128, KCH, 512], F32, tag="wi1x_xgB",
                               name="wi1xs")
            nc.sync.dma_start(
                wi1x_s[:], wi1x_d[:].rearrange("(k p) j -> p k j", p=128))
            hx = glob.tile([128, 4, QPAD], F32, tag="wi1", name="hx")
            impx_dram = dpool.tile([QPAD], F32, name="impx_dram")
            for mp in range(4):
                hps2 = ps.tile([128, QPAD], F32, tag="pp", name="hps2")
                for k in range(KCH):
                    nc.tensor.matmul(
                        out=hps2[:],
                        lhsT=wi1x_s[:, k, mp * 128:(mp + 1) * 128],
                        rhs=xqT[:, k, :],
                        start=(k == 0), stop=(k == KCH - 1),
                    )
                nc.scalar.activation(
                    out=hx[:, mp, :], in_=hps2[:], func=ACT.Relu,
                    bias=bi1_s[:, mp:mp + 1], scale=1.0,
                )
            ipx = ps.tile([1, QPAD], F32, tag="pp", name="ipx")
            for kp in range(4):
                nc.tensor.matmul(
                    out=ipx[:], lhsT=wi2x_s[:, kp:kp + 1],
                    rhs=hx[:, kp, :], start=(kp == 0), stop=(kp == 3),
                )
            impxb = glob.tile([1, QPAD], F32, tag="impxb")
            nc.scalar.copy(out=impxb[:], in_=ipx[:])
            nc.sync.dma_start(
                impx_dram[:].rearrange("(a b) -> a b", a=1), impxb[:])

            # ============ Phase C: merge, exact rescue, attended ============
            aT = glob.tile([128, KCH, QPAD], F32R, tag="btb_aT")
            if stages < 3:
                nc.vector.memset(aT[:], 0.0)
            with tc.tile_pool(name=f"pC{rep}", bufs=2) as pC, \
                 tc.tile_pool(name=f"pCm{rep}", bufs=1) as pCm, \
                 tc.tile_pool(name=f"xg{rep}", bufs=2) as xgp:
                for ci in range(NQCH if stages >= 3 else 0):
                    gidx = pCm.tile([128, 512], F32, tag="gidx")
                    nc.vector.tensor_copy(gidx[:], cidx[ci][:])   # u32->f32
                    nc.vector.tensor_tensor(out=gidx[:], in0=gidx[:],
                                            in1=cbase[:], op=AOP.add)
                    # mask own-message candidates
                    t0 = pCm.tile([128, 512], F32, tag="t0")
                    nc.vector.tensor_scalar(
                        out=t0[:], in0=gidx[:], scalar1=qlo[:, ci:ci + 1],
                        scalar2=None, op0=AOP.is_ge)
                    t1 = pCm.tile([128, 512], F32, tag="t1")
                    nc.vector.tensor_scalar(
                        out=t1[:], in0=gidx[:], scalar1=qhi[:, ci:ci + 1],
                        scalar2=None, op0=AOP.is_lt)
                    nc.vector.tensor_tensor(out=t0[:], in0=t0[:], in1=t1[:],
                                            op=AOP.mult)
                    cm = pCm.tile([128, 512], F32, tag="cm")
                    nc.vector.scalar_tensor_tensor(
                        out=cm[:], in0=t0[:], scalar=NEG, in1=cvals[ci][:],
                        op0=AOP.mult, op1=AOP.add,
                    )
                    # composite key: quantized score (9 bits) * 32768 + gidx
                    cq = pCm.tile([128, 512], F32, tag="cq")
                    nc.vector.tensor_scalar(
                        out=cq[:], in0=cm[:], scalar1=-8.0, scalar2=7.96875,
                        op0=AOP.max, op1=AOP.min)
                    nc.vector.tensor_scalar(
                        out=cq[:], in0=cq[:], scalar1=8.0, scalar2=32.0,
                        op0=AOP.add, op1=AOP.mult)
                    cqu = pCm.tile([128, 512], U32, tag="cqu")
                    nc.vector.tensor_copy(cqu[:], cq[:])
                    nc.vector.tensor_copy(cq[:], cqu[:])
                    comp = pCm.tile([128, 512], F32, tag="comp")
                    nc.vector.scalar_tensor_tensor(
                        out=comp[:], in0=cq[:], scalar=32768.0, in1=gidx[:],
                        op0=AOP.mult, op1=AOP.add,
                    )
                    # merged top-16 (fp8 screen noise ~0.05 + 1/32 quant can
                    # push a true-top-5 past rank 8 among the 512 candidates;
                    # measured 61/3264 at top-8, 0/3264 at top-16)
                    m16 = pC.tile([128, 16], F32, tag="m16")
                    nc.vector.max(out=m16[:, 0:8], in_=comp[:])
                    nc.vector.match_replace(
                        out=comp[:], in_to_replace=m16[:, 0:8],
                        in_values=comp[:], imm_value=NEG,
                    )
                    nc.vector.max(out=m16[:, 8:16], in_=comp[:])
                    # recover gidx = m16 mod 32768 (robust to round-vs-trunc)
                    dq = pC.tile([128, 16], F32, tag="dq")
                    nc.vector.tensor_scalar(
                        out=dq[:], in0=m16[:], scalar1=1.0 / 32768.0,
                        scalar2=None, op0=AOP.mult)
                    dqu = pC.tile([128, 16], U32, tag="dqu")
                    nc.vector.tensor_copy(dqu[:], dq[:])
                    nc.vector.tensor_copy(dq[:], dqu[:])
                    idx16f = pC.tile([128, 16], F32, tag="idx16f")
                    nc.vector.scalar_tensor_tensor(
                        out=idx16f[:], in0=dq[:], scalar=-32768.0, in1=m16[:],
                        op0=AOP.mult, op1=AOP.add,
                    )
                    neg = pC.tile([128, 16], F32, tag="negf")
                    nc.vector.tensor_scalar(
                        out=neg[:], in0=idx16f[:], scalar1=0.0, scalar2=32768.0,
                        op0=AOP.is_lt, op1=AOP.mult)
                    nc.vector.tensor_tensor(out=idx16f[:], in0=idx16f[:],
                                            in1=neg[:], op=AOP.add)
                    idx16u = pC.tile([128, 16], U32, tag="idx16u")
                    nc.vector.tensor_copy(idx16u[:], idx16f[:])

                    # gather candidate rows (f32, one indirect DMA per
                    # row -- multi-row offset APs abort on device) + exact
                    # fp32 rescore. Two 4-row groups live in the xg pool;
                    # two borrow the xqT and wi1x glob slots (dead once
                    # stage 2b's PE part ran).
                    xgg = [xgp.tile([128, 4, H], F32, tag=f"xgg{g}",
                                    name=f"xgg{g}") for g in range(2)]
                    xgg.append(glob.tile([128, 4, H], F32, tag="xqT",
                                         name=f"xgA{ci}"))
                    xgg.append(glob.tile([128, 4, H], F32, tag="wi1x_xgB",
                                         name=f"xgB{ci}"))
                    rows = [xgg[r // 4][:, r % 4, :] for r in range(16)]
                    s16 = pC.tile([128, 16], F32, tag="s16")
                    for r in range(16):
                        if sub >= 1:
                            nc.gpsimd.indirect_dma_start(
                                out=rows[r], out_offset=None, in_=x_d[:],
                                in_offset=bass.IndirectOffsetOnAxis(
                                    ap=idx16u[:, r:r + 1], axis=0),
                            )
                        else:
                            nc.vector.memset(rows[r], 0.5)
                        if sub >= 2:
                            scr = pC.tile([128, H], F32, tag="scr")
                            nc.vector.scalar_tensor_tensor(
                                out=scr[:], in0=b_nat[:, ci, :], scalar=1.0,
                                in1=rows[r], op0=AOP.mult, op1=AOP.mult,
                                accum_out=s16[:, r:r + 1],
                            )
                    if sub < 2:
                        nc.vector.tensor_copy(s16[:], m16[:])

                    # exact top-5 softmax over the 16 rescored candidates
                    srt8 = pC.tile([128, 8], F32, tag="srt8")
                    nc.vector.max(out=srt8[:], in_=s16[:])
                    nmax = pC.tile([128, 1], F32, tag="nmax")
                    nc.vector.tensor_scalar(
                        out=nmax[:], in0=srt8[:, 0:1], scalar1=-1.0,
                        scalar2=None, op0=AOP.mult)
                    e16 = pC.tile([128, 16], F32, tag="e16")
                    nc.scalar.activation(out=e16[:], in_=s16[:], func=ACT.Exp,
                                         bias=nmax[:, :1], scale=1.0)
                    msk = pC.tile([128, 16], F32, tag="msk")
                    nc.vector.tensor_scalar(
                        out=msk[:], in0=s16[:], scalar1=srt8[:, 4:5],
                        scalar2=None, op0=AOP.is_ge)
                    zsum = pC.tile([128, 1], F32, tag="zsum")
                    nc.vector.scalar_tensor_tensor(
                        out=e16[:], in0=e16[:], scalar=1.0, in1=msk[:],
                        op0=AOP.mult, op1=AOP.mult, accum_out=zsum[:, :1],
                    )
                    rz = pC.tile([128, 1], F32, tag="rz")
                    if sub >= 3:
                        nc.vector.reciprocal(out=rz[:], in_=zsum[:])
                    else:
                        nc.vector.memset(rz[:], 0.2)
                    attn = pC.tile([128, 16], F32, tag="attn")
                    nc.vector.tensor_scalar(
                        out=attn[:], in0=e16[:], scalar1=rz[:, :1],
                        scalar2=None, op0=AOP.mult)

                    # attended = sum_r attn_r * row_r ; transpose into aT
                    acc = pC.tile([128, H], F32, tag="acc")
                    nc.vector.tensor_scalar(
                        out=acc[:], in0=rows[0], scalar1=attn[:, 0:1],
                        scalar2=None, op0=AOP.mult)
                    for r in range(1, 16):
                        nc.vector.scalar_tensor_tensor(
                            out=acc[:], in0=rows[r], scalar=attn[:, r:r + 1],
                            in1=acc[:], op0=AOP.mult, op1=AOP.add,
                        )
                    for k in range(KCH):
                        tps = pst.tile([128, 128], F32, tag="tp")
                        nc.tensor.transpose(
                            out=tps[:], in_=acc[:, k * 128:(k + 1) * 128],
                            identity=ident[:],
                        )
                        nc.vector.tensor_copy(
                            aT[:, k, ci * 128:(ci + 1) * 128], tps[:])

            # ====== Stage 2b sort + sentinel export (DVE, tail) ======
            with tc.tile_pool(name=f"p2b{rep}", bufs=1) as p2b:
                vx = p2b.tile([8, 64], F32, tag="vx")
                nc.sync.dma_start(
                    vx[:], impx_dram[:].rearrange("(m t) -> m t", m=8))
                vxw = p2b.tile([8, 64], F32, tag="vxw")
                nc.vector.tensor_copy(vxw[:], vx[:])
                v8r = p2b.tile([8, 8], F32, tag="v8r")
                for r in range(6):
                    nc.vector.max(out=v8r[:], in_=vxw[:])
                    nc.vector.match_replace(
                        out=vxw[:], in_to_replace=v8r[:],
                        in_values=vxw[:], imm_value=NEG,
                    )
                nc.vector.max(out=v8r[:], in_=vxw[:])   # ranks 49..56
                # theta = exact 51st (col 2); valid = exact imp >= theta
                vm = p2b.tile([8, 64], F32, tag="vm")
                nc.vector.tensor_scalar(
                    out=vm[:], in0=vx[:], scalar1=v8r[:, 2:3], scalar2=None,
                    op0=AOP.is_ge)
                qsent = p2b.tile([8, 64], F32, tag="qsent")
                nc.vector.tensor_tensor(out=qsent[:], in0=qfg[:], in1=vm[:],
                                        op=AOP.mult)
                sent2 = p2b.tile([8, 64], F32, tag="sent2")
                nc.vector.tensor_scalar(
                    out=sent2[:], in0=vm[:], scalar1=-1.0e9, scalar2=1.0e9,
                    op0=AOP.mult, op1=AOP.add)
                nc.vector.tensor_tensor(out=qsent[:], in0=qsent[:],
                                        in1=sent2[:], op=AOP.add)
                qsu = p2b.tile([8, 64], U32, tag="qsu")
                nc.vector.tensor_copy(qsu[:], qsent[:])
                nc.sync.dma_start(
                    qidx_o[:].rearrange("(m t) -> m t", m=8), qsu[:])

            # ============ Phase D: output projections (f32r) ============
            if stages < 4:
                with tc.tile_pool(name=f"pDz{rep}", bufs=1) as pDz:
                    z = pDz.tile([128, KCH, QPAD], F32, tag="z")
                    nc.vector.memset(z[:], 0.0)
                    nc.sync.dma_start(
                        updT_o[:].rearrange("(k p) q -> p k q", p=128), z[:])
            with tc.tile_pool(name=f"pD{rep}", bufs=1) as pD:
                vT = pD.tile([128, KCH, QPAD], F32R, tag="vT")
                for mp in range(KCH if stages >= 4 else 0):
                    wsl = wslp.tile([128, KCH, 128], F32R, tag="wslr", name="wslr")
                    nc.sync.dma_start(
                        wsl[:], wv_r[:, :, mp * 128:(mp + 1) * 128])
                    vps = ps.tile([128, QPAD], F32, tag="pp")
                    for k in range(KCH):
                        nc.tensor.matmul(
                            out=vps[:], lhsT=wsl[:, k, :], rhs=aT[:, k, :],
                            start=(k == 0), stop=(k == KCH - 1),
                        )
                    nc.scalar.activation(
                        out=vT[:, mp, :], in_=vps[:], func=ACT.Identity,
                        bias=bv_s[:, mp:mp + 1], scale=1.0,
                    )
                upd = pD.tile([128, KCH, QPAD], F32, tag="upd")
                for mp in range(KCH if stages >= 4 else 0):
                    wsl = wslp.tile([128, KCH, 128], F32R, tag="wslr", name="wslr")
                    nc.sync.dma_start(
                        wsl[:], wo_r[:, :, mp * 128:(mp + 1) * 128])
                    ups = ps.tile([128, QPAD], F32, tag="pp")
                    for k in range(KCH):
                        nc.tensor.matmul(
                            out=ups[:], lhsT=wsl[:, k, :], rhs=vT[:, k, :],
                            start=(k == 0), stop=(k == KCH - 1),
                        )
                    nc.scalar.activation(
                        out=upd[:, mp, :], in_=ups[:], func=ACT.Identity,
                        bias=bo_s[:, mp:mp + 1], scale=1.0,
                    )
                if stages >= 4:
                    nc.sync.dma_start(
                        updT_o[:].rearrange("(k p) q -> p k q", p=128), upd[:])

    with tile.TileContext(nc) as tc:
        for rep in range(reps):
            emit_body(tc, rep)

    nc.compile()
    return nc


def _host_prep(inputs):
    import ml_dtypes

    X = np.ascontiguousarray(np.asarray(inputs["token_features"],
                                        dtype=np.float32))
    B = np.asarray(inputs["message_boundaries"]).astype(np.int64)
    starts, ends = B[:, 0], B[:, 1]

    XT = np.ascontiguousarray(X.T)                     # [H, T] f32
    XT8 = np.clip(XT, -240.0, 240.0).astype(ml_dtypes.float8_e4m3fn)

    w = {k: np.ascontiguousarray(np.asarray(inputs[k], dtype=np.float32))
         for k in ("Wq", "Wk", "Wv", "Wo", "Wi1")}
    b = {k: np.asarray(inputs[k], dtype=np.float32)
         for k in ("bq", "bk", "bv", "bo", "bi1", "bi2")}
    Wi2 = np.asarray(inputs["Wi2"], dtype=np.float32)  # [512, 1]

    # tokens t with t2m[t]==m form [ends[m-1], ends[m])
    mlo = np.concatenate([[0], ends[:-1]]).astype(np.float32)
    mhi = ends.astype(np.float32)

    common = {
        "x": X,
        "xt8": XT8,
        "wqk": np.ascontiguousarray(
            (w["Wq"].astype(np.float64) @ w["Wk"].T.astype(np.float64)
             / 32.0).astype(np.float32)),
        "wv": w["Wv"],
        "wo": w["Wo"],
        "wi1b": w["Wi1"].astype(ml_dtypes.bfloat16),
        "wi1x": w["Wi1"],
        "wi2s": np.ascontiguousarray(Wi2[:, 0].reshape(4, 128).T),
        "wi2r": np.ascontiguousarray(Wi2[:, 0].reshape(4, 128).T),
        "bi1s": np.ascontiguousarray(b["bi1"].reshape(4, 128).T),
        "bvs": np.ascontiguousarray(b["bv"].reshape(8, 128).T),
        "bos": np.ascontiguousarray(b["bo"].reshape(8, 128).T),
        "cbase": np.ascontiguousarray(
            np.tile(((np.arange(512) // 8) * 512).astype(np.float32)[None, :],
                    (128, 1))),
    }

    in_maps = []
    for c in range(NCORES):
        msgs = np.arange(c * MSG_PER_CORE, (c + 1) * MSG_PER_CORE)
        row_m = np.repeat(msgs, QPAD // MSG_PER_CORE)   # [512], 64 per msg
        qlo_row = mlo[row_m].astype(np.float32)
        qhi_row = mhi[row_m].astype(np.float32)
        m = dict(common)
        m["xtcb"] = np.ascontiguousarray(
            XT[:, c * TC:(c + 1) * TC]).astype(ml_dtypes.bfloat16)
        m["bstg"] = starts[msgs].astype(np.float32).reshape(8, 1)
        m["qlo"] = np.ascontiguousarray(qlo_row.reshape(4, 128).T)
        m["qhi"] = np.ascontiguousarray(qhi_row.reshape(4, 128).T)
        in_maps.append(m)
    return in_maps


def _numpy_fallback(inputs):
    """Reference semantics in numpy (only for non-equal-length boundaries,
    which the stated harness never produces)."""
    X = np.asarray(inputs["token_features"], dtype=np.float32)
    B = np.asarray(inputs["message_boundaries"]).astype(np.int64)
    imp = (np.maximum(X @ np.asarray(inputs["Wi1"]) + np.asarray(inputs["bi1"]),
                      0) @ np.asarray(inputs["Wi2"])
           + np.asarray(inputs["bi2"]))[:, 0]
    k_imp = max(1, int((T // M) * 0.1))
    impm = imp.reshape(M, T // M)
    top_local = np.argsort(-impm, axis=1, kind="stable")[:, :k_imp]
    qidx_raw = (top_local + B[:, 0:1]).reshape(-1)
    qidx = np.minimum(qidx_raw, T - 1)
    Qp = X[qidx] @ np.asarray(inputs["Wq"]) + np.asarray(inputs["bq"])
    Km = X @ np.asarray(inputs["Wk"]) + np.asarray(inputs["bk"])
    S = (Qp @ Km.T) / np.float32(32.0)
    t2m = np.searchsorted(B[:, 1], np.arange(T), side="right")
    msg_ids = np.repeat(np.arange(M), k_imp)
    S[msg_ids[:, None] == t2m[None, :]] = -np.inf
    k_conn = min(5, T // M)
    top_idx = np.argsort(-S, axis=1, kind="stable")[:, :k_conn]
    tv = np.take_along_axis(S, top_idx, axis=1)
    e = np.exp(tv - tv[:, :1])
    attn = e / e.sum(1, keepdims=True)
    V = X @ np.asarray(inputs["Wv"]) + np.asarray(inputs["bv"])
    att = np.einsum("qk,qkh->qh", attn.astype(np.float32), V[top_idx])
    upd = att @ np.asarray(inputs["Wo"]) + np.asarray(inputs["bo"])
    out = X.copy()
    ok = qidx_raw < T
    np.add.at(out, qidx_raw[ok], upd[ok].astype(np.float32))
    return out


def kernel(**inputs):
    X = np.asarray(inputs["token_features"])
    B = np.asarray(inputs["message_boundaries"]).astype(np.int64)
    assert X.shape == (T, H), X.shape

    eq = (np.array_equal(B[:, 0], np.arange(M) * L)
          and np.array_equal(B[:, 1], (np.arange(M) + 1) * L))
    if not eq or np.any(np.asarray(inputs["bq"])):
        # bq != 0 would invalidate the host-fused W_qk = Wq WkT / 32
        return _numpy_fallback(inputs)

    from concourse.bass_utils import run_bass_kernel_spmd

    if "nc" not in _cache:
        _cache["nc"] = _build_program(
            int(os.environ.get("KERNEL_STAGES", "99")),
            int(os.environ.get("KERNEL_SUB", "9")),
            int(os.environ.get("KERNEL_REPS", "1")))
    nc = _cache["nc"]

    in_maps = _host_prep(inputs)
    trace = bool(int(os.environ.get("KERNEL_PROFILE", "0")))
    res = run_bass_kernel_spmd(nc, in_maps, list(range(NCORES)), trace=trace)
    if trace:
        _cache["exec_time_ns"] = res.exec_time_ns
        _cache["mean_exec_time_ns"] = res.mean_exec_time_ns

    out = np.array(X, dtype=np.float32, copy=True)
    idx_parts, upd_parts = [], []
    for c in range(NCORES):
        qi = res.results[c]["qidx_o"].astype(np.int64)
        valid = qi < T
        idx_parts.append(qi[valid])
        upd_parts.append(res.results[c]["updT_o"].T[valid].astype(np.float32))
    all_idx = np.concatenate(idx_parts)
    all_upd = np.concatenate(upd_parts)
    if len(np.unique(all_idx)) == len(all_idx):
        out[all_idx] += all_upd
    else:
        np.add.at(out, all_idx, all_upd)
    return out
